# revision 21
# baseline (speedup 1.0000x reference)
"""Trainium2 Bass kernel for nn_BlockMoEAdapters (8 NeuronCores, SPMD).

Sharding: tokens (B*N = 4096) split contiguously across 8 cores (512 each).
Cores 0-3 hold batch 0, cores 4-7 batch 1. Attention K/V are all-gathered
(bf16, per half: k then v, launched as soon as each half's GEMM finishes so
the gather hides behind the q GEMM) within each 4-core batch group; MoE
capacity ranks use a tiny 8-core all-gather of per-core expert counts.

Layout: channel-major ([channels, tokens]) on-device for all GEMMs; LayerNorm
stats via ones-matmul partition reductions, rsqrt via ln/exp on the scalar
engine (keeps a single natural_log_exp activation table until the gelu
phase); softmax in [keys, tokens] orientation with the denominator from a
ones-column appended to V; denominator rows are gathered into [8, TL] tiles
(DVE copy + sbuf-to-sbuf DMA), reciprocated in one batched DVE op per group
of 4 pairs, and broadcast back per pair with a selector matmul; attention is
software-pipelined one beat ahead with the two heads of a pair issued to
disjoint PE row groups so their K=64 score matmuls run concurrently. Weights
host-retiled into per-output-slab layouts; output re-transposed on host.
"""
import sys

for _p in ('/opt/trn_rl_repo',):
    if _p not in sys.path:
        sys.path.append(_p)

import ml_dtypes
import numpy as np

import concourse.bass as bass
import concourse.mybir as mybir
import concourse.tile as tile
from concourse import bacc
from concourse.bass_utils import run_bass_kernel_spmd

F32 = mybir.dt.float32
F32R = mybir.dt.float32r
BF16 = mybir.dt.bfloat16
FP8 = mybir.dt.float8e4
I8 = mybir.dt.int8
AF = mybir.ActivationFunctionType
ALU = mybir.AluOpType
DR = mybir.MatmulPerfMode.DoubleRow
# Schraudolph exp in fp8-e4m3 bit domain: bits = round(z/ln2*8 + 7*8 + c)
# for weight = exp(0.125*score); z = 0.125*score folded into the scale.
SCH_A = 0.125 * 8.0 / np.log(2.0)
SCH_B = 56.0 - 0.344 + 0.5   # +0.5 centers the truncating float->int cast

B, N, D = 2, 2048, 1024
H, HD = 16, 64
E, TOPK = 4, 2
MOEH, MLPH = 256, 4096
T = B * N
NC = 8
TL = T // NC          # 512 tokens per core
NT = TL // 128        # 4 token tiles
DT = D // 128         # 8 channel tiles
CAP = int(T * TOPK / E * 1.0)   # 2048
GRP = 4               # cores per kv-gather group
EPS = 1e-5

_cache = {}


def _mm(nc, out, lhsT, rhs, start, stop, dt=None):
    if dt is not None:
        lhsT, rhs = lhsT.bitcast(dt), rhs.bitcast(dt)
    nc.tensor.matmul(out, lhsT, rhs, start=start, stop=stop)


def _build():
    nc = bacc.Bacc("TRN2", target_bir_lowering=False, debug=False,
                   num_devices=NC)

    def din(name, shape, dt=F32):
        return nc.dram_tensor(name, list(shape), dt, kind="ExternalInput")

    xT_d = din("xT", (D, TL), BF16)
    noiseT_d = din("noiseT", (E, TL), BF16)
    # host-retiled weight slabs (see _prep_inputs for layouts)
    wqk_d = din("wqk_l", (128, 16 * DT * 128), FP8)
    wv_d = din("wv_l", (128, DT * 1024), FP8)
    wproj_d = din("wproj_l", (128, DT * DT * 128), FP8)
    wmlp1_d = din("wmlp1_l", (128, 32 * DT * 128), BF16)
    we1_d = din("we1_l", (128, 8 * DT * 128), BF16)
    wout_d = din("wout_l", (128, DT * 40 * 128), BF16)
    wrn_d = din("wrn", (128, DT * 2 * E), BF16)   # w_route || w_noise tiles
    be2_d = din("be2", (E, D))
    # column-tiled small constants: [128, n] with col j = chunk j
    ln1g_d = din("ln1g", (128, DT))
    ln1b_d = din("ln1b", (128, DT))
    ln2g_d = din("ln2g", (128, DT))
    ln2b_d = din("ln2b", (128, DT))
    bproj_d = din("bproj", (128, DT))
    be1_d = din("be1", (128, E * MOEH // 128))
    bmlp1_d = din("bmlp1", (128, MLPH // 128))
    bmlp2_d = din("bmlp2", (128, DT))
    brb_d = din("brb", (E, 2))               # broute | bnoise columns
    ones_d = din("ones128", (128, 128))
    onesb_d = din("ones128b", (128, 1), BF16)
    eye_d = din("eye128", (128, 128))
    utri_d = din("utri128", (128, 128))      # U[s,t] = 1 if s < t
    gsel_d = din("gsel", (E, E * 128))       # gsel[k, e*128+p] = (k == e)
    gsel8_d = din("gsel8", (8, 4 * 128))     # gsel8[h, q*128+c] = (h==2q+c//64)
    wpfx_d = din("wpfx", (NC, 1))            # per-core: 1 for j < core_id

    out_d = nc.dram_tensor("out", [D, TL], F32, kind="ExternalOutput")

    rg_kv = [[0, 1, 2, 3], [4, 5, 6, 7]]
    rg_all = [list(range(NC))]

    with tile.TileContext(nc) as tc:
        with (
            tc.tile_pool(name="dram", bufs=1, space="DRAM") as dpool,
            tc.tile_pool(name="consts", bufs=1) as cpool,
            tc.tile_pool(name="persist", bufs=1) as ppool,
            # PSUM: 's2' 3x[128,1024] = 6 banks, 'ao' 2x[128,512] = 2 banks
            tc.tile_pool(name="ps_s2", bufs=3, space="PSUM") as ps_s2,
            tc.tile_pool(name="ps_ao", bufs=2, space="PSUM") as ps_ao,
            tc.tile_pool(name="wslab", bufs=2) as wpool,
            tc.tile_pool(name="scratch", bufs=2) as spool,
        ):
            def big_ps(name):
                # [128, TL] psum tile in an 's2'-class slot
                return ps_s2.tile([128, TL], F32, tag="s2", name=name)

            def misc_ps(shape, name):
                return ps_ao.tile(list(shape), F32, tag="ao", name=name)

            # ---------- collective bounce buffers (k / v per half) --------
            # kept under 1 MB per message so the collective stays on the
            # fast mesh algorithm (>=1 MB switches to ring, ~2.5x slower);
            # v travels in fp8 (it feeds the fp8 DoubleRow AV matmul anyway)
            k_in = [dpool.tile([D // 2, TL], FP8, name=f"k_in{h_}")
                    for h_ in range(2)]
            v_in = [dpool.tile([128, 4 * 640], FP8, name=f"v_in{h_}")
                    for h_ in range(2)]
            k_out = [dpool.tile([GRP, D // 2, TL], FP8, name=f"k_out{h_}")
                     for h_ in range(2)]
            v_out = [dpool.tile([GRP * 128, 4 * 640], FP8,
                                name=f"v_out{h_}") for h_ in range(2)]
            cnt_in = dpool.tile([1, E], F32, name="cnt_in")
            cnt_out = dpool.tile([NC, E], F32, name="cnt_out")

            # ---------- constants (gpsimd queue, one DMA each) ----------
            def load_const(dram, shape, dt=F32, tag=None):
                tag = tag or dram.name
                t = cpool.tile(list(shape), dt, tag=tag, name=tag)
                nc.gpsimd.dma_start(t[:], dram[:])
                return t

            ones_sb = load_const(ones_d, (128, 128))
            onesr_sb = load_const(ones_d, (128, 128), F32R, tag="ones128r")
            onesb_sb = load_const(onesb_d, (128, 1), BF16)
            gsel_sb = load_const(gsel_d, (E, E * 128), F32R)
            gsel8_sb = load_const(gsel8_d, (8, 4 * 128), F32R)
            eye_sb = load_const(eye_d, (128, 128))
            utri_sb = load_const(utri_d, (128, 128))
            wpfx_sb = load_const(wpfx_d, (NC, 1))
            brb_sb = load_const(brb_d, (E, 2))
            be2_sb = load_const(be2_d, (E, D), F32R)
            ln1g_sb = load_const(ln1g_d, (128, DT))
            ln1b_sb = load_const(ln1b_d, (128, DT))
            ln2g_sb = load_const(ln2g_d, (128, DT))
            ln2b_sb = load_const(ln2b_d, (128, DT))
            bproj_sb = load_const(bproj_d, (128, DT))
            be1_sb = load_const(be1_d, (128, E * MOEH // 128))
            bmlp1_sb = load_const(bmlp1_d, (128, MLPH // 128))
            bmlp2_sb = load_const(bmlp2_d, (128, DT))
            wrn_sb = load_const(wrn_d, (128, DT * 2 * E), BF16)

            # ---------- load x (CM, bf16 for GEMM-side) ----------
            xT_sb = []
            for j in range(DT):
                t = ppool.tile([128, TL], BF16, tag=f"xT{j}", name=f"xT{j}")
                nc.sync.dma_start(t[:], xT_d[j * 128:(j + 1) * 128, :])
                xT_sb.append(t)

            epsc = cpool.tile([1, 1], F32, tag="epsc", name="epsc")
            nc.vector.memset(epsc[:], EPS)

            # ---------- LayerNorm in CM; bf16 in/out tiles ----------
            def layernorm_cm(xtiles, g_sb, b_sb, outtag, out_aps=None):
                ones_col = onesb_sb[:, 0:1]
                musum = misc_ps((1, TL), "musum")
                sqsum = misc_ps((1, TL), "sqsum")
                for j in range(DT):
                    sq = spool.tile([128, TL], BF16, tag="lnsq", name="lnsq",
                                    bufs=2)
                    nc.vector.tensor_tensor(sq[:], xtiles[j][:], xtiles[j][:],
                                            ALU.mult)
                    _mm(nc, musum[:], ones_col, xtiles[j][:],
                        j == 0, j == DT - 1)
                    _mm(nc, sqsum[:], ones_col, sq[:],
                        j == 0, j == DT - 1)
                mu = spool.tile([1, TL], F32R, tag="lnmu", name="lnmu", bufs=1)
                nc.vector.tensor_scalar_mul(mu[:], musum[:], 1.0 / D)
                msq = spool.tile([1, TL], F32, tag="lnscr", name="lnmsq",
                                 bufs=2)
                nc.vector.tensor_tensor(msq[:], mu[:].bitcast(F32),
                                        mu[:].bitcast(F32), ALU.mult)
                var = spool.tile([1, TL], F32, tag="lnscr", name="lnvar",
                                 bufs=2)
                nc.vector.scalar_tensor_tensor(var[:], sqsum[:], 1.0 / D,
                                               msq[:], ALU.mult, ALU.subtract)
                # rsqrt(var+eps) = exp(-0.5*ln(var+eps)) (stays in nle set)
                lnv = spool.tile([1, TL], F32, tag="lnscr", name="lnlnv",
                                 bufs=2)
                nc.scalar.activation(lnv[:], var[:], AF.Ln, bias=epsc[:])
                rsig = spool.tile([1, TL], F32R, tag="lnrsig", name="lnrsig",
                                  bufs=1)
                nc.scalar.activation(rsig[:], lnv[:], AF.Exp, scale=-0.5)
                mub_ps = misc_ps((128, TL), "mub")
                _mm(nc, mub_ps[:], onesr_sb[0:1, :], mu[:], True, True)
                rsb_ps = misc_ps((128, TL), "rsb")
                _mm(nc, rsb_ps[:], onesr_sb[0:1, :], rsig[:], True, True)
                outs = []
                for j in range(DT):
                    t1 = spool.tile([128, TL], F32, tag="lnt1", name="lnt1",
                                    bufs=2)
                    nc.vector.tensor_tensor(t1[:], xtiles[j][:], mub_ps[:],
                                            ALU.subtract)
                    t2 = spool.tile([128, TL], F32, tag="lnt2", name="lnt2",
                                    bufs=2)
                    nc.vector.tensor_tensor(t2[:], t1[:], rsb_ps[:], ALU.mult)
                    if out_aps is None:
                        o = ppool.tile([128, TL], BF16, tag=f"{outtag}{j}",
                                       name=f"{outtag}{j}")
                        oa = o[:]
                    else:
                        o, oa = None, out_aps[j]
                    nc.scalar.activation(oa, t2[:], AF.Identity,
                                         bias=b_sb[:, j:j + 1],
                                         scale=g_sb[:, j:j + 1])
                    outs.append(o)
                return outs

            qT_sb = [ppool.tile([128, TL], FP8, tag=f"qT{m}", name=f"qT{m}")
                     for m in range(DT)]

            x1a = ppool.tile([128, DT * TL], FP8, tag="x1all",
                             name="x1all")
            with tc.tile_pool(name="st1", bufs=2) as s1pool:
                layernorm_cm(xT_sb, ln1g_sb, ln1b_sb, "x1T",
                             out_aps=[x1a[:, j * TL:(j + 1) * TL]
                                      for j in range(DT)])
                x1f = x1a[:].rearrange("p (k x) -> p k x", x=TL)

                def qk_gemm(m):
                    # m 0-7: q slabs; 8-15: k slabs (fp8 DoubleRow over
                    # adjacent contraction-tile pairs)
                    slab = wpool.tile([128, DT * 128], FP8, tag="qkslab8",
                                      name="qkslab")
                    nc.sync.dma_start(
                        slab[:], wqk_d[:, m * 1024:(m + 1) * 1024])
                    ps = big_ps("qk")
                    sl = slab[:].rearrange("p (k c) -> p k c", c=128)
                    for kk in range(DT):
                        nc.tensor.matmul(ps[:], sl[:, kk, :], x1f[:, kk, :],
                                         start=kk == 0, stop=kk == DT - 1)
                    if m < DT:
                        nc.vector.tensor_copy(qT_sb[m][:], ps[:])
                    else:
                        ksb = s1pool.tile([128, TL], FP8, tag="kevac",
                                          name="kevac", bufs=2)
                        nc.vector.tensor_copy(ksb[:], ps[:])
                        mk = m - DT
                        nc.gpsimd.dma_start(
                            k_in[mk // 4][(mk % 4) * 128:(mk % 4 + 1) * 128,
                                          :], ksb[:])

                def v_gemm(nn):
                    # v output channels nn*512 .. (nn+1)*512, TM layout + pad
                    wv_slabs = []
                    for j in range(DT // 2):
                        s = s1pool.tile([128, 2 * 512], FP8, tag=f"wv{j}",
                                        name=f"wv{j}", bufs=1)
                        for i in range(2):
                            kk = 2 * j + i
                            nc.sync.dma_start(
                                s[:, i * 512:(i + 1) * 512],
                                wv_d[:, kk * 1024 + nn * 512:
                                     kk * 1024 + (nn + 1) * 512])
                        wv_slabs.append(s)
                    for mt in range(NT):
                        ps = big_ps("vps")
                        for kk in range(DT):
                            nc.tensor.matmul(
                                ps[:],
                                x1f[:, kk, mt * 128:(mt + 1) * 128],
                                wv_slabs[kk // 2][:, (kk % 2) * 512:
                                                  (kk % 2 + 1) * 512],
                                start=kk == 0, stop=kk == DT - 1)
                        vp = s1pool.tile([128, 640], FP8, tag="vpad",
                                         name="vpad", bufs=2)
                        nc.vector.memset(vp[:], 1.0)
                        dst = vp[:].rearrange("p (h c) -> p h c", c=80)
                        nc.vector.tensor_copy(
                            dst[:, :, 0:64],
                            ps[:].rearrange("p (h c) -> p h c", c=64))
                        nc.gpsimd.dma_start(
                            v_in[nn][:].rearrange(
                                "p (q c) -> p q c",
                                c=640)[:, :, mt * 160:(mt + 1) * 160],
                            vp[:].rearrange("p (q c) -> p q c", c=160))

                def ag(buf_in, buf_out):
                    nc.gpsimd.collective_compute(
                        "AllGather", ALU.bypass, replica_groups=rg_kv,
                        ins=[buf_in[:].opt()], outs=[buf_out[:].opt()])

                # all k/v first so the serial CC chain starts ASAP; q last
                # (q is only needed when attention starts)
                for m in (8, 9, 10, 11):
                    qk_gemm(m)
                ag(k_in[0], k_out[0])
                v_gemm(0)
                ag(v_in[0], v_out[0])
                for m in (12, 13, 14, 15):
                    qk_gemm(m)
                ag(k_in[1], k_out[1])
                v_gemm(1)
                ag(v_in[1], v_out[1])
                for m in range(DT):       # q
                    qk_gemm(m)

            # ---------- attention (pipelined beats, row-packed heads) -----
            # ao reuses the x1 slot (dead once the qkv GEMMs finish);
            # fp8 so proj can run as DoubleRow
            ao_a = ppool.tile([128, DT * TL], FP8, tag="x1all",
                              name="ao_all")   # pair p cols: rows 0:64 head 2p
            with (
                tc.tile_pool(name="attn", bufs=2) as apool,
                tc.tile_pool(name="vsb", bufs=2) as vpool,
                tc.tile_pool(name="ssb", bufs=4) as spool_s,
            ):
                # denominators for pairs 0-3 / 4-7 (partition = head mod 8)
                den = [ppool.tile([8, TL], F32, tag=f"den{g}",
                                  name=f"den{g}") for g in range(2)]

                def den_finish(g):
                    # reciprocal + per-pair broadcast + normalize
                    r8 = spool_s.tile([8, TL], F32R, tag="r8", name="r8",
                                      bufs=1)
                    with nc.allow_low_precision(reason="f32r recip for bcast"):
                        nc.vector.reciprocal(r8[:], den[g][:])
                    for q in range(4):
                        p = 4 * g + q
                        bcp = misc_ps((128, TL), "dbc")
                        _mm(nc, bcp[:], gsel8_sb[:, q * 128:(q + 1) * 128],
                            r8[:], True, True)
                        nc.vector.tensor_tensor(
                            ao_a[:, p * TL:(p + 1) * TL],
                            ao_a[:, p * TL:(p + 1) * TL], bcp[:], ALU.mult)

                for p in range(DT):              # head pair
                    hf, pq = p // 4, p % 4       # kv half, pair in half
                    kp = []
                    vt = []
                    for r in range(GRP):
                        kt_ = apool.tile([128, TL], FP8, tag=f"kp{r}",
                                         name=f"kp{r}")
                        nc.sync.dma_start(
                            kt_[:], k_out[hf][r, pq * 128:(pq + 1) * 128, :])
                        kp.append(kt_)
                        vt_ = vpool.tile([128, 640], FP8, tag=f"vt{r}",
                                         name=f"vt{r}")
                        nc.sync.dma_start(
                            vt_[:],
                            v_out[hf][r * 128:(r + 1) * 128,
                                      pq * 640:(pq + 1) * 640])
                        vt.append(vt_)
                    ao_ps = [ps_ao.tile([65, TL], F32, tag="ao",
                                        name=f"ao{hh}") for hh in range(2)]
                    ssb = {}

                    def scores(beat):
                        s2 = [ps_s2.tile([128, 2 * TL], F32, tag="s2",
                                         name=f"s2_{hh}") for hh in range(2)]
                        # interleave heads so the K=64 matmuls land in
                        # disjoint PE row groups and run concurrently
                        for u in range(2):
                            kt = 2 * beat + u
                            r, cc = kt // 4, kt % 4
                            for hh in range(2):
                                po = 64 * hh
                                _mm(nc, s2[hh][:, u * TL:(u + 1) * TL],
                                    kp[r][po:po + 64,
                                          cc * 128:(cc + 1) * 128],
                                    qT_sb[p][po:po + 64, :], True, True)
                        for hh in range(2):
                            # softmax exp -> fp8 weights; head 0 exact on the
                            # scalar engine, head 1 via the Schraudolph
                            # exp-in-bit-domain trick on the (idle) DVE so the
                            # two run concurrently
                            s_sb = spool_s.tile([128, 2 * TL], FP8,
                                                tag="ssb", name="ssb")
                            if hh == 0:
                                nc.scalar.activation(s_sb[:], s2[hh][:],
                                                     AF.Exp, scale=0.125)
                            else:
                                nc.vector.tensor_scalar(
                                    s_sb[:].bitcast(I8), s2[hh][:],
                                    SCH_A, SCH_B, ALU.mult, ALU.add)
                            ssb[(beat, hh)] = s_sb

                    def avs(beat):
                        # fp8 DoubleRow: one matmul per head folds both key
                        # tiles (consecutive cc on the same partitions)
                        cc = (2 * beat) % 4
                        r = (2 * beat) // 4
                        for hh in range(2):
                            s_sb = ssb.pop((beat, hh))
                            sv = s_sb[:].rearrange("p (u x) -> p u x", x=TL)
                            vv = vt[r][:].rearrange(
                                "p (c x) -> p c x",
                                x=160)[:, cc:cc + 2, 80 * hh:80 * hh + 65]
                            nc.tensor.matmul(ao_ps[hh][:], vv, sv,
                                             start=beat == 0, stop=beat == 7,
                                             perf_mode=DR)

                    scores(0)
                    for beat in range(1, 8):
                        scores(beat)
                        avs(beat - 1)
                    avs(7)

                    # evacuate unnormalized ao + stash denominator rows
                    for hh in range(2):
                        po = 64 * hh
                        nc.scalar.activation(
                            ao_a[po:po + 64, p * TL:(p + 1) * TL],
                            ao_ps[hh][0:64, :], AF.Copy)
                        dcp = spool_s.tile([1, TL], F32, tag="dcp",
                                           name="dcp", bufs=2)
                        nc.vector.tensor_copy(dcp[:], ao_ps[hh][64:65, :])
                        h8 = 2 * pq + hh
                        nc.sync.dma_start(den[hf][h8:h8 + 1, :], dcp[:])
                    if p == 3:
                        den_finish(0)
                den_finish(1)

                # ---------- proj + residual ----------
                xres = []
                xres_bf = []
                aof = ao_a[:].rearrange("p (k x) -> p k x", x=TL)
                for m in range(DT):
                    slab = wpool.tile([128, DT * 128], FP8, tag="qkslab8",
                                      name="projslab")
                    nc.sync.dma_start(
                        slab[:], wproj_d[:, m * 1024:(m + 1) * 1024])
                    ps = big_ps("proj")
                    sl = slab[:].rearrange("p (k c) -> p k c", c=128)
                    for kk in range(DT):
                        nc.tensor.matmul(ps[:], sl[:, kk, :], aof[:, kk, :],
                                         start=kk == 0, stop=kk == DT - 1)
                    xr = ppool.tile([128, TL], F32, tag=f"xres{m}",
                                    name=f"xres{m}")
                    nc.vector.scalar_tensor_tensor(
                        xr[:], ps[:], bproj_sb[:, m:m + 1], xT_sb[m][:],
                        ALU.add, ALU.add)
                    xb = ppool.tile([128, TL], BF16, tag=f"xresb{m}",
                                    name=f"xresb{m}")
                    nc.vector.tensor_copy(xb[:], xr[:])
                    xres.append(xr)
                    xres_bf.append(xb)

            # ---------- LN2 (x2T reuses the qT slots, dead after attn) ----
            x2T = layernorm_cm(xres_bf, ln2g_sb, ln2b_sb, "x2T")

            # router + gates scheduled at high priority so their
            # exp ops land before the gelu table switch
            with tc.high_priority():
                # ---------- router (route | noise fused GEMM) ----------
                rt_ps = misc_ps((E, TL), "rt")
                for j in range(DT):
                    _mm(nc, rt_ps[:],
                        wrn_sb[:, j * 2 * E:j * 2 * E + E],
                        x2T[j][:], j == 0, j == DT - 1)
                nn_ps = misc_ps((E, TL), "nn")
                for j in range(DT):
                    _mm(nc, nn_ps[:],
                        wrn_sb[:, j * 2 * E + E:(j + 1) * 2 * E],
                        x2T[j][:], j == 0, j == DT - 1)
                logits = spool.tile([E, TL], F32, tag="logits", name="logits",
                                    bufs=1)
                nc.vector.tensor_scalar(logits[:], rt_ps[:],
                                        brb_sb[:, 0:1], None, ALU.add)
                spe = spool.tile([E, TL], BF16, tag="softpe", name="softpe",
                                 bufs=1)
                nc.scalar.activation(spe[:], nn_ps[:], AF.Exp,
                                     bias=brb_sb[:, 1:2])
                spe1 = spool.tile([E, TL], BF16, tag="softpe1",
                                  name="softpe1", bufs=1)
                nc.vector.tensor_scalar_add(spe1[:], spe[:], 1.0)
                sp = spool.tile([E, TL], BF16, tag="softp", name="softp",
                                bufs=1)
                nc.scalar.activation(sp[:], spe1[:], AF.Ln)
                noiseT_sb = spool.tile([E, TL], BF16, tag="noiseTs",
                                       name="noiseTs", bufs=1)
                nc.sync.dma_start(noiseT_sb[:], noiseT_d[:])
                nsp = spool.tile([E, TL], BF16, tag="nsp", name="nsp", bufs=1)
                nc.vector.tensor_tensor(nsp[:], noiseT_sb[:], sp[:], ALU.mult)
                noisy_cm = spool.tile([E, TL], F32, tag="noisycm", name="noisycm",
                                      bufs=1)
                nc.vector.tensor_tensor(noisy_cm[:], nsp[:], logits[:], ALU.add)

                # ---------- top-2 gates (TM); single batched exp ----------
                noisy8 = ppool.tile([128, 8 * NT], F32, tag="noisy8",
                                    name="noisy8")
                nc.vector.memset(noisy8[:], -1e30)
                m8 = ppool.tile([128, 8 * NT], F32, tag="m8", name="m8")
                gate = ppool.tile([128, E * NT], F32, tag="gate", name="gate")
                mask = ppool.tile([128, E * NT], F32, tag="mask", name="mask")
                geT = ppool.tile([E, TL], F32R, tag="geT", name="geT")
                cnt_sb = ppool.tile([1, NT * E], F32, tag="cntsb", name="cntsb")
                for j in range(NT):
                    tr_ps = misc_ps((128, E), "ntr")
                    nc.tensor.matmul(tr_ps[:],
                                     noisy_cm[:, j * 128:(j + 1) * 128],
                                     eye_sb[0:E, 0:E], is_transpose=True,
                                     start=True, stop=True)
                    nc.vector.tensor_copy(noisy8[:, 8 * j:8 * j + E], tr_ps[:])
                for j in range(NT):
                    nc.vector.max(m8[:, 8 * j:8 * j + 8],
                                  noisy8[:, 8 * j:8 * j + 8])
                m8v = m8[:].rearrange("p (j c) -> p j c", c=8)
                dall = spool.tile([128, NT], F32, tag="dall", name="dall",
                                  bufs=1)
                nc.vector.tensor_tensor(dall[:], m8v[:, :, 1], m8v[:, :, 0],
                                        ALU.subtract)
                # sigma(d) = 1/(1+exp(-d)) -- uses the exp table already
                # resident from attention (tanh would force a table swap
                # between the gelu loads)
                emd = spool.tile([128, NT], F32, tag="th", name="emd", bufs=1)
                nc.scalar.activation(emd[:], dall[:], AF.Exp, scale=-1.0)
                ope = spool.tile([128, NT], F32, tag="ope", name="ope", bufs=1)
                nc.vector.tensor_scalar_add(ope[:], emd[:], 1.0)
                spos = spool.tile([128, NT], F32, tag="spos", name="spos",
                                  bufs=1)
                nc.vector.reciprocal(spos[:], ope[:])
                sneg = spool.tile([128, NT], F32, tag="sneg", name="sneg",
                                  bufs=1)
                nc.vector.tensor_scalar(sneg[:], spos[:], -1.0, 1.0, ALU.mult,
                                        ALU.add)
                for j in range(NT):
                    nm = noisy8[:, 8 * j:8 * j + E]
                    v1 = m8[:, 8 * j:8 * j + 1]
                    v2 = m8[:, 8 * j + 1:8 * j + 2]
                    oh1 = spool.tile([128, E], F32, tag="oh1", name="oh1")
                    nc.vector.tensor_scalar(oh1[:], nm, v1, None, ALU.is_ge)
                    msk = mask[:, E * j:E * (j + 1)]
                    nc.vector.tensor_scalar(msk, nm, v2, None, ALU.is_ge)
                    oh2 = spool.tile([128, E], F32, tag="oh2", name="oh2")
                    nc.vector.tensor_tensor(oh2[:], msk, oh1[:], ALU.subtract)
                    g1 = spool.tile([128, E], F32, tag="gnum", name="g1")
                    nc.vector.tensor_scalar(g1[:], oh1[:],
                                            sneg[:, j:j + 1], None, ALU.mult)
                    g2 = spool.tile([128, E], F32, tag="gnum2", name="g2")
                    nc.vector.tensor_scalar(g2[:], oh2[:],
                                            spos[:, j:j + 1], None, ALU.mult)
                    nc.vector.tensor_tensor(gate[:, E * j:E * (j + 1)],
                                            g1[:], g2[:], ALU.add)
                    cps = misc_ps((1, E), "cnt")
                    _mm(nc, cps[:], ones_sb[:, 0:1], msk, True, True, F32)
                    nc.vector.tensor_copy(cnt_sb[0:1, E * j:E * (j + 1)], cps[:])

                # total counts -> all-gather
                tot = spool.tile([1, E], F32, tag="cnttot", name="cnttot",
                                 bufs=1)
                nc.vector.tensor_tensor(tot[:], cnt_sb[0:1, 0:E],
                                        cnt_sb[0:1, E:2 * E], ALU.add)
                nc.vector.tensor_tensor(tot[:], tot[:], cnt_sb[0:1, 2 * E:3 * E],
                                        ALU.add)
                nc.vector.tensor_tensor(tot[:], tot[:], cnt_sb[0:1, 3 * E:4 * E],
                                        ALU.add)
                nc.sync.dma_start(cnt_in[:], tot[:])
                nc.gpsimd.collective_compute(
                    "AllGather", ALU.bypass, replica_groups=rg_all,
                    ins=[cnt_in[:].opt()], outs=[cnt_out[:].opt()])

            # ---------- MLP hidden + MoE hidden (overlaps counts AG) ------
            Hm_sb = []
            for m in range(MLPH // 128):
                slab = wpool.tile([128, DT * 128], BF16, tag="qkslab",
                                  name="m1slab")
                nc.sync.dma_start(
                    slab[:], wmlp1_d[:, m * 1024:(m + 1) * 1024])
                ps = big_ps("hm")
                for kk in range(DT):
                    _mm(nc, ps[:], slab[:, kk * 128:(kk + 1) * 128],
                        x2T[kk][:], kk == 0, kk == DT - 1)
                hm = ppool.tile([128, TL], BF16, tag=f"hm{m}", name=f"hm{m}")
                nc.scalar.activation(hm[:], ps[:], AF.Gelu,
                                     bias=bmlp1_sb[:, m:m + 1])
                Hm_sb.append(hm)
            Hmoe = []
            for e in range(E):
                for hmi in range(MOEH // 128):
                    me = 2 * e + hmi
                    slab = wpool.tile([128, DT * 128], BF16, tag="qkslab",
                                      name="e1slab")
                    nc.sync.dma_start(
                        slab[:], we1_d[:, me * 1024:(me + 1) * 1024])
                    ps = big_ps("hmoe")
                    for kk in range(DT):
                        _mm(nc, ps[:], slab[:, kk * 128:(kk + 1) * 128],
                            x2T[kk][:], kk == 0, kk == DT - 1)
                    hs = ppool.tile([128, TL], BF16, tag=f"hmoe{me}",
                                    name=f"hmoe{me}")
                    nc.scalar.activation(
                        hs[:], ps[:], AF.Gelu,
                        bias=be1_sb[:, me:me + 1])
                    Hmoe.append(hs)

            # ---------- ranks / keep / gate_eff ----------
            cntg = spool.tile([NC, E], F32, tag="cntg", name="cntg", bufs=1)
            nc.sync.dma_start(cntg[:], cnt_out[:])
            off_ps = misc_ps((1, E), "off")
            _mm(nc, off_ps[:], wpfx_sb[:], cntg[:], True, True, F32)
            car = spool.tile([1, E * NT], F32, tag="car", name="car", bufs=1)
            nc.vector.tensor_copy(car[:, 0:E], off_ps[:])
            for j in range(1, NT):
                nc.vector.tensor_tensor(car[:, E * j:E * (j + 1)],
                                        car[:, E * (j - 1):E * j],
                                        cnt_sb[0:1, E * (j - 1):E * j],
                                        ALU.add)
            ge_tm = ppool.tile([128, E * NT], F32, tag="getm", name="getm")
            for j in range(NT):
                rk_ps = misc_ps((128, E), "rank")
                _mm(nc, rk_ps[:], utri_sb[:],
                    mask[:, E * j:E * (j + 1)], True, False, F32)
                _mm(nc, rk_ps[:], ones_sb[0:1, :],
                    car[:, E * j:E * (j + 1)], False, True, F32)
                keep = spool.tile([128, E], F32, tag="keep", name="keep")
                nc.vector.tensor_scalar(keep[:], rk_ps[:], float(CAP), None,
                                        ALU.is_lt)
                nc.vector.tensor_tensor(ge_tm[:, E * j:E * (j + 1)],
                                        gate[:, E * j:E * (j + 1)],
                                        keep[:], ALU.mult)
            for j in range(NT):
                tr_ps = misc_ps((E, 128), "getr")
                nc.tensor.matmul(tr_ps[:], ge_tm[:, E * j:E * (j + 1)],
                                 eye_sb[:, :], is_transpose=True,
                                 start=True, stop=True)
                nc.vector.tensor_copy(geT[:, j * 128:(j + 1) * 128], tr_ps[:])

            # gate the MoE hidden
            Hg = []
            for e in range(E):
                bc_ps = misc_ps((128, TL), "gbc")
                _mm(nc, bc_ps[:], gsel_sb[:, e * 128:(e + 1) * 128],
                    geT[:], True, True)
                bc_sb = spool.tile([128, TL], BF16, tag="gbcsb", name="gbcsb",
                                   bufs=2)
                nc.vector.tensor_copy(bc_sb[:], bc_ps[:])
                for hmi in range(MOEH // 128):
                    hg = ppool.tile([128, TL], BF16, tag=f"hg{2*e+hmi}",
                                    name=f"hg{2*e+hmi}")
                    nc.vector.tensor_tensor(hg[:], Hmoe[2 * e + hmi][:],
                                            bc_sb[:], ALU.mult)
                    Hg.append(hg)

            # ---------- output GEMM: moe + be2 + mlp, fused accum ----------
            for m in range(DT):
                slab = wpool.tile([128, 40 * 128], BF16, tag="outslab",
                                  name="outslab")
                nc.sync.dma_start(
                    slab[:], wout_d[:, m * 5120:(m + 1) * 5120])
                ps = big_ps("out")
                for kk in range(MLPH // 128):   # gate-independent part first
                    _mm(nc, ps[:],
                        slab[:, (8 + kk) * 128:(9 + kk) * 128],
                        Hm_sb[kk][:], kk == 0, False)
                for i8 in range(8):          # we2 tiles (e, hmi)
                    _mm(nc, ps[:], slab[:, i8 * 128:(i8 + 1) * 128],
                        Hg[i8][:], False, False)
                _mm(nc, ps[:], be2_sb[:, m * 128:(m + 1) * 128],
                    geT[:], False, True)
                o = spool.tile([128, TL], F32, tag="outsb", name="outsb",
                               bufs=2)
                nc.vector.scalar_tensor_tensor(
                    o[:], ps[:], bmlp2_sb[:, m:m + 1], xres[m][:],
                    ALU.add, ALU.add)
                nc.sync.dma_start(out_d[m * 128:(m + 1) * 128, :], o[:])

    nc.compile()
    return nc


def _tile_lhst(w, n_k, n_m):
    # w: [n_k*128, n_m*128] -> [128, n_m, n_k, 128] -> [128, n_m*n_k*128]
    kdim, mdim = w.shape
    return np.ascontiguousarray(
        w.reshape(n_k, 128, n_m, 128).transpose(1, 2, 0, 3)
        .reshape(128, n_m * n_k * 128))


def _cols(a, n):
    # [n*128] -> [128, n] with column j = chunk j
    return np.ascontiguousarray(
        np.asarray(a, np.float32).reshape(n, 128).T)


def _prep_inputs(inputs):
    f32 = lambda a: np.ascontiguousarray(np.asarray(a, np.float32))
    bf = lambda a: np.ascontiguousarray(
        np.asarray(a, np.float32).astype(ml_dtypes.bfloat16))
    f8 = lambda a: np.ascontiguousarray(
        np.asarray(a, np.float32).astype(ml_dtypes.float8_e4m3))
    x = f32(inputs["x"]).reshape(T, D)
    noise = f32(inputs["noise"]).reshape(T, E)
    w_qkv = np.asarray(inputs["w_qkv"], np.float32)
    wqkT = w_qkv[:2 * D].T                       # [D, 2048]
    wvT = w_qkv[2 * D:].T                        # [D, D]
    wprojT = np.asarray(inputs["w_proj"], np.float32).T
    we1 = np.asarray(inputs["we1"], np.float32)  # [E, D, MOEH]
    we2 = np.asarray(inputs["we2"], np.float32)  # [E, MOEH, D]
    wmlp1 = np.asarray(inputs["w_mlp1"], np.float32)   # [D, MLPH]
    wmlp2 = np.asarray(inputs["w_mlp2"], np.float32)   # [MLPH, D]

    # we1 slabs: m-index = e*2+hmi over [D, 256] each
    we1_flat = np.concatenate([we1[e] for e in range(E)], 1)  # [D, E*MOEH]
    # wout: per m, 8 we2 tiles (e,hmi) then 32 wmlp2 tiles
    we2_l = we2.reshape(E, 2, 128, DT, 128).transpose(2, 3, 0, 1, 4) \
        .reshape(128, DT, 8, 128)
    wm2_l = wmlp2.reshape(32, 128, DT, 128).transpose(1, 2, 0, 3)
    wout = np.concatenate([we2_l, wm2_l], 2).reshape(128, DT * 40 * 128)

    shared = dict(
        wqk_l=f8(_tile_lhst(wqkT, DT, 16)),
        wv_l=f8(np.ascontiguousarray(
            wvT.reshape(DT, 128, D).transpose(1, 0, 2).reshape(128, DT * D))),
        wproj_l=f8(_tile_lhst(wprojT, DT, DT)),
        wmlp1_l=bf(_tile_lhst(wmlp1, DT, 32)),
        we1_l=bf(_tile_lhst(we1_flat, DT, 8)),
        wout_l=bf(wout),
        wrn=bf(np.concatenate([f32(inputs["w_route"]),
                               f32(inputs["w_noise"])], 1)
               .reshape(DT, 128, 2 * E).transpose(1, 0, 2)
               .reshape(128, DT * 2 * E)),
        be2=f32(inputs["be2"]),
        ln1g=_cols(inputs["ln1_g"], DT),
        ln1b=_cols(inputs["ln1_b"], DT),
        ln2g=_cols(inputs["ln2_g"], DT),
        ln2b=_cols(inputs["ln2_b"], DT),
        bproj=_cols(inputs["b_proj"], DT),
        be1=_cols(inputs["be1"], E * MOEH // 128),
        bmlp1=_cols(inputs["b_mlp1"], MLPH // 128),
        bmlp2=_cols(inputs["b_mlp2"], DT),
        brb=np.ascontiguousarray(np.stack(
            [f32(inputs["b_route"]), f32(inputs["b_noise"])], 1)),
        ones128=np.ones((128, 128), np.float32),
        eye128=np.eye(128, dtype=np.float32),
        utri128=np.triu(np.ones((128, 128), np.float32), 1),
        gsel=np.repeat(np.eye(E, dtype=np.float32), 128, 1),
        gsel8=np.ascontiguousarray((np.arange(8)[:, None] == (
            2 * (np.arange(512) // 128) + (np.arange(512) % 128) // 64
        )[None, :]).astype(np.float32)),
        ones128b=np.ones((128, 1), ml_dtypes.bfloat16),
    )
    in_maps = []
    for c in range(NC):
        m = dict(shared)
        m["xT"] = bf(x[c * TL:(c + 1) * TL].T)
        m["noiseT"] = bf(noise[c * TL:(c + 1) * TL].T)
        m["wpfx"] = (np.arange(NC) < c).astype(np.float32).reshape(NC, 1)
        in_maps.append(m)
    return in_maps


def _run(inputs, trace=False):
    if "nc" not in _cache:
        _cache["nc"] = _build()
    nc = _cache["nc"]
    in_maps = _prep_inputs(inputs)
    res = run_bass_kernel_spmd(nc, in_maps, core_ids=list(range(NC)),
                               trace=trace)
    _cache["last_res"] = res
    shards = [res.results[c]["out"] for c in range(NC)]   # each [D, TL]
    out = np.concatenate([np.asarray(s, np.float32).T for s in shards],
                         0).reshape(B, N, D)
    return out.astype(np.float32), res.exec_time_ns


def kernel(**inputs):
    out, _ = _run(inputs, trace=False)
    return out



# revision 22
# speedup vs baseline: 1.4486x; 1.4486x over previous
"""Trainium2 Bass kernel for nn_BlockMoEAdapters (8 NeuronCores, SPMD).

Sharding: tokens (B*N = 4096) split contiguously across 8 cores (512 each).
Cores 0-3 hold batch 0, cores 4-7 batch 1. Attention K/V are all-gathered
(bf16, per half: k then v, launched as soon as each half's GEMM finishes so
the gather hides behind the q GEMM) within each 4-core batch group; MoE
capacity ranks use a tiny 8-core all-gather of per-core expert counts.

Layout: channel-major ([channels, tokens]) on-device for all GEMMs; LayerNorm
stats via ones-matmul partition reductions, rsqrt via ln/exp on the scalar
engine (keeps a single natural_log_exp activation table until the gelu
phase); softmax in [keys, tokens] orientation with the denominator from a
ones-column appended to V; denominator rows are gathered into [8, TL] tiles
(DVE copy + sbuf-to-sbuf DMA), reciprocated in one batched DVE op per group
of 4 pairs, and broadcast back per pair with a selector matmul; attention is
software-pipelined one beat ahead with the two heads of a pair issued to
disjoint PE row groups so their K=64 score matmuls run concurrently. Weights
host-retiled into per-output-slab layouts; output re-transposed on host.
"""
import sys

for _p in ('/opt/trn_rl_repo',):
    if _p not in sys.path:
        sys.path.append(_p)

import ml_dtypes
import numpy as np

import concourse.bass as bass
import concourse.mybir as mybir
import concourse.tile as tile
from concourse import bacc
from concourse.bass_utils import run_bass_kernel_spmd

F32 = mybir.dt.float32
F32R = mybir.dt.float32r
BF16 = mybir.dt.bfloat16
FP8 = mybir.dt.float8e4
I8 = mybir.dt.int8
AF = mybir.ActivationFunctionType
ALU = mybir.AluOpType
DR = mybir.MatmulPerfMode.DoubleRow
# Schraudolph exp in fp8-e4m3 bit domain: bits = round(z/ln2*8 + 7*8 + c)
# for weight = exp(0.125*score); z = 0.125*score folded into the scale.
SCH_A = 0.125 * 8.0 / np.log(2.0)
SCH_B = 56.0 - 0.344 + 0.5   # +0.5 centers the truncating float->int cast

B, N, D = 2, 2048, 1024
H, HD = 16, 64
E, TOPK = 4, 2
MOEH, MLPH = 256, 4096
T = B * N
NC = 8
TL = T // NC          # 512 tokens per core
NT = TL // 128        # 4 token tiles
DT = D // 128         # 8 channel tiles
CAP = int(T * TOPK / E * 1.0)   # 2048
GRP = 4               # cores per kv-gather group
EPS = 1e-5

_cache = {}


def _mm(nc, out, lhsT, rhs, start, stop, dt=None):
    if dt is not None:
        lhsT, rhs = lhsT.bitcast(dt), rhs.bitcast(dt)
    nc.tensor.matmul(out, lhsT, rhs, start=start, stop=stop)


def _build():
    nc = bacc.Bacc("TRN2", target_bir_lowering=False, debug=False,
                   num_devices=NC)

    def din(name, shape, dt=F32):
        return nc.dram_tensor(name, list(shape), dt, kind="ExternalInput")

    xT_d = din("xT", (D, TL), BF16)
    noiseT_d = din("noiseT", (E, TL), BF16)
    # host-retiled weight slabs (see _prep_inputs for layouts)
    wqk_d = din("wqk_l", (128, 16 * DT * 128), FP8)
    wv_d = din("wv_l", (128, DT * 1024), FP8)
    wproj_d = din("wproj_l", (128, DT * DT * 128), FP8)
    wmlp1_d = din("wmlp1_l", (128, 32 * DT * 128), BF16)
    we1_d = din("we1_l", (128, 8 * DT * 128), BF16)
    wout_d = din("wout_l", (128, DT * 40 * 128), BF16)
    wrn_d = din("wrn", (128, DT * 2 * E), BF16)   # w_route || w_noise tiles
    be2_d = din("be2", (E, D))
    # column-tiled small constants: [128, n] with col j = chunk j
    ln1g_d = din("ln1g", (128, DT))
    ln1b_d = din("ln1b", (128, DT))
    ln2g_d = din("ln2g", (128, DT))
    ln2b_d = din("ln2b", (128, DT))
    bproj_d = din("bproj", (128, DT))
    be1_d = din("be1", (128, E * MOEH // 128))
    bmlp1_d = din("bmlp1", (128, MLPH // 128))
    bmlp2_d = din("bmlp2", (128, DT))
    brb_d = din("brb", (E, 2))               # broute | bnoise columns
    ones_d = din("ones128", (128, 128))
    onesb_d = din("ones128b", (128, 1), BF16)
    eye_d = din("eye128", (128, 128))
    utri_d = din("utri128", (128, 128))      # U[s,t] = 1 if s < t
    gsel_d = din("gsel", (E, E * 128))       # gsel[k, e*128+p] = (k == e)
    gsel8_d = din("gsel8", (8, 4 * 128))     # gsel8[h, q*128+c] = (h==2q+c//64)
    wpfx_d = din("wpfx", (NC, 1))            # per-core: 1 for j < core_id

    out_d = nc.dram_tensor("out", [D, TL], F32, kind="ExternalOutput")

    rg_kv = [[0, 1, 2, 3], [4, 5, 6, 7]]
    rg_all = [list(range(NC))]

    with tile.TileContext(nc) as tc:
        with (
            tc.tile_pool(name="dram", bufs=1, space="DRAM") as dpool,
            tc.tile_pool(name="consts", bufs=1) as cpool,
            tc.tile_pool(name="persist", bufs=1) as ppool,
            # PSUM: 's2' 3x[128,1024] = 6 banks, 'ao' 2x[128,512] = 2 banks
            tc.tile_pool(name="ps_s2", bufs=3, space="PSUM") as ps_s2,
            tc.tile_pool(name="ps_ao", bufs=2, space="PSUM") as ps_ao,
            tc.tile_pool(name="wslab", bufs=2) as wpool,
            tc.tile_pool(name="scratch", bufs=2) as spool,
        ):
            def big_ps(name):
                # [128, TL] psum tile in an 's2'-class slot
                return ps_s2.tile([128, TL], F32, tag="s2", name=name)

            def misc_ps(shape, name):
                return ps_ao.tile(list(shape), F32, tag="ao", name=name)

            # ---------- collective bounce buffers (k / v per half) --------
            # kept under 1 MB per message so the collective stays on the
            # fast mesh algorithm (>=1 MB switches to ring, ~2.5x slower);
            # v travels in fp8 (it feeds the fp8 DoubleRow AV matmul anyway)
            KSZ = (D // 2) * TL
            VSZ = 128 * 4 * 640
            KVSZ = KSZ + VSZ          # 576 KB < 1 MB keeps the mesh algo
            kv_in = [dpool.tile([1, KVSZ], FP8, name=f"kv_in{h_}")
                     for h_ in range(2)]
            kv_out = [dpool.tile([GRP, KVSZ], FP8, name=f"kv_out{h_}")
                      for h_ in range(2)]
            cnt_in = dpool.tile([1, E], F32, name="cnt_in")
            cnt_out = dpool.tile([NC, E], F32, name="cnt_out")

            # ---------- constants (gpsimd queue, one DMA each) ----------
            def load_const(dram, shape, dt=F32, tag=None):
                tag = tag or dram.name
                t = cpool.tile(list(shape), dt, tag=tag, name=tag)
                nc.gpsimd.dma_start(t[:], dram[:])
                return t

            ones_sb = load_const(ones_d, (128, 128))
            onesr_sb = load_const(ones_d, (128, 128), F32R, tag="ones128r")
            onesb_sb = load_const(onesb_d, (128, 1), BF16)
            gsel_sb = load_const(gsel_d, (E, E * 128), F32R)
            gsel8_sb = load_const(gsel8_d, (8, 4 * 128), F32R)
            eye_sb = load_const(eye_d, (128, 128))
            utri_sb = load_const(utri_d, (128, 128))
            wpfx_sb = load_const(wpfx_d, (NC, 1))
            brb_sb = load_const(brb_d, (E, 2))
            be2_sb = load_const(be2_d, (E, D), F32R)
            ln1g_sb = load_const(ln1g_d, (128, DT))
            ln1b_sb = load_const(ln1b_d, (128, DT))
            ln2g_sb = load_const(ln2g_d, (128, DT))
            ln2b_sb = load_const(ln2b_d, (128, DT))
            bproj_sb = load_const(bproj_d, (128, DT))
            be1_sb = load_const(be1_d, (128, E * MOEH // 128))
            bmlp1_sb = load_const(bmlp1_d, (128, MLPH // 128))
            bmlp2_sb = load_const(bmlp2_d, (128, DT))
            wrn_sb = load_const(wrn_d, (128, DT * 2 * E), BF16)

            # ---------- load x (CM, bf16 for GEMM-side) ----------
            xT_sb = []
            for j in range(DT):
                t = ppool.tile([128, TL], BF16, tag=f"xT{j}", name=f"xT{j}")
                nc.sync.dma_start(t[:], xT_d[j * 128:(j + 1) * 128, :])
                xT_sb.append(t)

            epsc = cpool.tile([1, 1], F32, tag="epsc", name="epsc")
            nc.vector.memset(epsc[:], EPS)

            # ---------- LayerNorm in CM; bf16 in/out tiles ----------
            def layernorm_cm(xtiles, g_sb, b_sb, outtag, out_aps=None):
                ones_col = onesb_sb[:, 0:1]
                musum = misc_ps((1, TL), "musum")
                sqsum = misc_ps((1, TL), "sqsum")
                for j in range(DT):
                    sq = spool.tile([128, TL], BF16, tag="lnsq", name="lnsq",
                                    bufs=2)
                    nc.vector.tensor_tensor(sq[:], xtiles[j][:], xtiles[j][:],
                                            ALU.mult)
                    _mm(nc, musum[:], ones_col, xtiles[j][:],
                        j == 0, j == DT - 1)
                    _mm(nc, sqsum[:], ones_col, sq[:],
                        j == 0, j == DT - 1)
                mu = spool.tile([1, TL], F32R, tag="lnmu", name="lnmu", bufs=1)
                nc.vector.tensor_scalar_mul(mu[:], musum[:], 1.0 / D)
                msq = spool.tile([1, TL], F32, tag="lnscr", name="lnmsq",
                                 bufs=2)
                nc.vector.tensor_tensor(msq[:], mu[:].bitcast(F32),
                                        mu[:].bitcast(F32), ALU.mult)
                var = spool.tile([1, TL], F32, tag="lnscr", name="lnvar",
                                 bufs=2)
                nc.vector.scalar_tensor_tensor(var[:], sqsum[:], 1.0 / D,
                                               msq[:], ALU.mult, ALU.subtract)
                # rsqrt(var+eps) = exp(-0.5*ln(var+eps)) (stays in nle set)
                lnv = spool.tile([1, TL], F32, tag="lnscr", name="lnlnv",
                                 bufs=2)
                nc.scalar.activation(lnv[:], var[:], AF.Ln, bias=epsc[:])
                rsig = spool.tile([1, TL], F32R, tag="lnrsig", name="lnrsig",
                                  bufs=1)
                nc.scalar.activation(rsig[:], lnv[:], AF.Exp, scale=-0.5)
                mub_ps = misc_ps((128, TL), "mub")
                _mm(nc, mub_ps[:], onesr_sb[0:1, :], mu[:], True, True)
                rsb_ps = misc_ps((128, TL), "rsb")
                _mm(nc, rsb_ps[:], onesr_sb[0:1, :], rsig[:], True, True)
                outs = []
                for j in range(DT):
                    t1 = spool.tile([128, TL], F32, tag="lnt1", name="lnt1",
                                    bufs=2)
                    nc.vector.tensor_tensor(t1[:], xtiles[j][:], mub_ps[:],
                                            ALU.subtract)
                    t2 = spool.tile([128, TL], F32, tag="lnt2", name="lnt2",
                                    bufs=2)
                    nc.vector.tensor_tensor(t2[:], t1[:], rsb_ps[:], ALU.mult)
                    if out_aps is None:
                        o = ppool.tile([128, TL], BF16, tag=f"{outtag}{j}",
                                       name=f"{outtag}{j}")
                        oa = o[:]
                    else:
                        o, oa = None, out_aps[j]
                    nc.scalar.activation(oa, t2[:], AF.Identity,
                                         bias=b_sb[:, j:j + 1],
                                         scale=g_sb[:, j:j + 1])
                    outs.append(o)
                return outs

            qT_sb = [ppool.tile([128, TL], FP8, tag=f"qT{m}", name=f"qT{m}")
                     for m in range(DT)]

            x1a = ppool.tile([128, DT * TL], FP8, tag="x1all",
                             name="x1all")
            with tc.tile_pool(name="st1", bufs=2) as s1pool:
                layernorm_cm(xT_sb, ln1g_sb, ln1b_sb, "x1T",
                             out_aps=[x1a[:, j * TL:(j + 1) * TL]
                                      for j in range(DT)])
                x1f = x1a[:].rearrange("p (k x) -> p k x", x=TL)

                def qk_gemm(m):
                    # m 0-7: q slabs; 8-15: k slabs (fp8 DoubleRow over
                    # adjacent contraction-tile pairs)
                    slab = wpool.tile([128, DT * 128], FP8, tag="qkslab8",
                                      name="qkslab")
                    nc.sync.dma_start(
                        slab[:], wqk_d[:, m * 1024:(m + 1) * 1024])
                    ps = big_ps("qk")
                    sl = slab[:].rearrange("p (k c) -> p k c", c=128)
                    for kk in range(DT):
                        nc.tensor.matmul(ps[:], sl[:, kk, :], x1f[:, kk, :],
                                         start=kk == 0, stop=kk == DT - 1)
                    if m < DT:
                        nc.vector.tensor_copy(qT_sb[m][:], ps[:])
                    else:
                        ksb = s1pool.tile([128, TL], FP8, tag="kevac",
                                          name="kevac", bufs=2)
                        nc.vector.tensor_copy(ksb[:], ps[:])
                        mk = m - DT
                        nc.gpsimd.dma_start(
                            kv_in[mk // 4][0:1, (mk % 4) * 128 * TL:
                                           (mk % 4 + 1) * 128 * TL], ksb[:])

                def v_gemm(nn):
                    # v output channels nn*512 .. (nn+1)*512, TM layout + pad
                    wv_slabs = []
                    for j in range(DT // 2):
                        s = s1pool.tile([128, 2 * 512], FP8, tag=f"wv{j}",
                                        name=f"wv{j}", bufs=1)
                        for i in range(2):
                            kk = 2 * j + i
                            nc.sync.dma_start(
                                s[:, i * 512:(i + 1) * 512],
                                wv_d[:, kk * 1024 + nn * 512:
                                     kk * 1024 + (nn + 1) * 512])
                        wv_slabs.append(s)
                    for mt in range(NT):
                        ps = big_ps("vps")
                        for kk in range(DT):
                            nc.tensor.matmul(
                                ps[:],
                                x1f[:, kk, mt * 128:(mt + 1) * 128],
                                wv_slabs[kk // 2][:, (kk % 2) * 512:
                                                  (kk % 2 + 1) * 512],
                                start=kk == 0, stop=kk == DT - 1)
                        vp = s1pool.tile([128, 640], FP8, tag="vpad",
                                         name="vpad", bufs=2)
                        nc.vector.memset(vp[:], 1.0)
                        dst = vp[:].rearrange("p (h c) -> p h c", c=80)
                        nc.vector.tensor_copy(
                            dst[:, :, 0:64],
                            ps[:].rearrange("p (h c) -> p h c", c=64))
                        nc.gpsimd.dma_start(
                            kv_in[nn][0:1, KSZ:KSZ + VSZ].rearrange(
                                "a (p q c) -> a p q c", q=4,
                                c=640)[:, :, :, mt * 160:(mt + 1) * 160],
                            vp[:].rearrange("p (q c) -> p q c", c=160))

                def ag(buf_in, buf_out):
                    nc.gpsimd.collective_compute(
                        "AllGather", ALU.bypass, replica_groups=rg_kv,
                        ins=[buf_in[:].opt()], outs=[buf_out[:].opt()])

                # all k/v first so the serial CC chain starts ASAP; q last
                # (q is only needed when attention starts)
                for m in (8, 9, 10, 11):
                    qk_gemm(m)
                v_gemm(0)
                ag(kv_in[0], kv_out[0])
                for m in (12, 13, 14, 15):
                    qk_gemm(m)
                v_gemm(1)
                ag(kv_in[1], kv_out[1])
                for m in range(DT):       # q
                    qk_gemm(m)

            # ---------- attention (pipelined beats, row-packed heads) -----
            # ao reuses the x1 slot (dead once the qkv GEMMs finish);
            # fp8 so proj can run as DoubleRow
            ao_a = ppool.tile([128, DT * TL], FP8, tag="x1all",
                              name="ao_all")   # pair p cols: rows 0:64 head 2p
            with (
                tc.tile_pool(name="attn", bufs=2) as apool,
                tc.tile_pool(name="vsb", bufs=2) as vpool,
                tc.tile_pool(name="ssb", bufs=4) as spool_s,
            ):
                # denominators for pairs 0-3 / 4-7 (partition = head mod 8)
                den = [ppool.tile([8, TL], F32, tag=f"den{g}",
                                  name=f"den{g}") for g in range(2)]

                def den_finish(g):
                    # reciprocal + per-pair broadcast + normalize
                    r8 = spool_s.tile([8, TL], F32R, tag="r8", name="r8",
                                      bufs=1)
                    with nc.allow_low_precision(reason="f32r recip for bcast"):
                        nc.vector.reciprocal(r8[:], den[g][:])
                    for q in range(4):
                        p = 4 * g + q
                        bcp = misc_ps((128, TL), "dbc")
                        _mm(nc, bcp[:], gsel8_sb[:, q * 128:(q + 1) * 128],
                            r8[:], True, True)
                        nc.vector.tensor_tensor(
                            ao_a[:, p * TL:(p + 1) * TL],
                            ao_a[:, p * TL:(p + 1) * TL], bcp[:], ALU.mult)

                for p in range(DT):              # head pair
                    hf, pq = p // 4, p % 4       # kv half, pair in half
                    kp = []
                    vt = []
                    for r in range(GRP):
                        kt_ = apool.tile([128, TL], FP8, tag=f"kp{r}",
                                         name=f"kp{r}")
                        nc.sync.dma_start(
                            kt_[:],
                            kv_out[hf][r:r + 1,
                                       pq * 128 * TL:(pq + 1) * 128 * TL]
                            .rearrange("a (p c) -> a p c", c=TL))
                        kp.append(kt_)
                        vt_ = vpool.tile([128, 640], FP8, tag=f"vt{r}",
                                         name=f"vt{r}")
                        nc.sync.dma_start(
                            vt_[:],
                            kv_out[hf][r:r + 1, KSZ:KSZ + VSZ].rearrange(
                                "a (p q c) -> a p q c", q=4,
                                c=640)[:, :, pq, :])
                        vt.append(vt_)
                    ao_ps = [ps_ao.tile([65, TL], F32, tag="ao",
                                        name=f"ao{hh}") for hh in range(2)]
                    ssb = {}

                    def scores(beat):
                        s2 = [ps_s2.tile([128, 2 * TL], F32, tag="s2",
                                         name=f"s2_{hh}") for hh in range(2)]
                        # interleave heads so the K=64 matmuls land in
                        # disjoint PE row groups and run concurrently
                        for u in range(2):
                            kt = 2 * beat + u
                            r, cc = kt // 4, kt % 4
                            for hh in range(2):
                                po = 64 * hh
                                _mm(nc, s2[hh][:, u * TL:(u + 1) * TL],
                                    kp[r][po:po + 64,
                                          cc * 128:(cc + 1) * 128],
                                    qT_sb[p][po:po + 64, :], True, True)
                        for hh in range(2):
                            # softmax exp -> fp8 weights; head 0 exact on the
                            # scalar engine, head 1 via the Schraudolph
                            # exp-in-bit-domain trick on the (idle) DVE so the
                            # two run concurrently
                            s_sb = spool_s.tile([128, 2 * TL], FP8,
                                                tag="ssb", name="ssb")
                            if hh == 0:
                                nc.scalar.activation(s_sb[:], s2[hh][:],
                                                     AF.Exp, scale=0.125)
                            else:
                                nc.vector.tensor_scalar(
                                    s_sb[:].bitcast(I8), s2[hh][:],
                                    SCH_A, SCH_B, ALU.mult, ALU.add)
                            ssb[(beat, hh)] = s_sb

                    def avs(beat):
                        # fp8 DoubleRow: one matmul per head folds both key
                        # tiles (consecutive cc on the same partitions)
                        cc = (2 * beat) % 4
                        r = (2 * beat) // 4
                        for hh in range(2):
                            s_sb = ssb.pop((beat, hh))
                            sv = s_sb[:].rearrange("p (u x) -> p u x", x=TL)
                            vv = vt[r][:].rearrange(
                                "p (c x) -> p c x",
                                x=160)[:, cc:cc + 2, 80 * hh:80 * hh + 65]
                            nc.tensor.matmul(ao_ps[hh][:], vv, sv,
                                             start=beat == 0, stop=beat == 7,
                                             perf_mode=DR)

                    scores(0)
                    for beat in range(1, 8):
                        scores(beat)
                        avs(beat - 1)
                    avs(7)

                    # evacuate unnormalized ao + stash denominator rows
                    for hh in range(2):
                        po = 64 * hh
                        nc.scalar.activation(
                            ao_a[po:po + 64, p * TL:(p + 1) * TL],
                            ao_ps[hh][0:64, :], AF.Copy)
                        dcp = spool_s.tile([1, TL], F32, tag="dcp",
                                           name="dcp", bufs=2)
                        nc.vector.tensor_copy(dcp[:], ao_ps[hh][64:65, :])
                        h8 = 2 * pq + hh
                        nc.sync.dma_start(den[hf][h8:h8 + 1, :], dcp[:])
                    if p == 3:
                        den_finish(0)
                den_finish(1)

                # ---------- proj + residual ----------
                xres = []
                xres_bf = []
                aof = ao_a[:].rearrange("p (k x) -> p k x", x=TL)
                for m in range(DT):
                    slab = wpool.tile([128, DT * 128], FP8, tag="qkslab8",
                                      name="projslab")
                    nc.sync.dma_start(
                        slab[:], wproj_d[:, m * 1024:(m + 1) * 1024])
                    ps = big_ps("proj")
                    sl = slab[:].rearrange("p (k c) -> p k c", c=128)
                    for kk in range(DT):
                        nc.tensor.matmul(ps[:], sl[:, kk, :], aof[:, kk, :],
                                         start=kk == 0, stop=kk == DT - 1)
                    xr = ppool.tile([128, TL], F32, tag=f"xres{m}",
                                    name=f"xres{m}")
                    nc.vector.scalar_tensor_tensor(
                        xr[:], ps[:], bproj_sb[:, m:m + 1], xT_sb[m][:],
                        ALU.add, ALU.add)
                    xb = ppool.tile([128, TL], BF16, tag=f"xresb{m}",
                                    name=f"xresb{m}")
                    nc.vector.tensor_copy(xb[:], xr[:])
                    xres.append(xr)
                    xres_bf.append(xb)

            # ---------- LN2 (x2T reuses the qT slots, dead after attn) ----
            x2T = layernorm_cm(xres_bf, ln2g_sb, ln2b_sb, "x2T")

            # router + gates scheduled at high priority so their
            # exp ops land before the gelu table switch
            with tc.high_priority():
                # ---------- router (route | noise fused GEMM) ----------
                rt_ps = misc_ps((E, TL), "rt")
                for j in range(DT):
                    _mm(nc, rt_ps[:],
                        wrn_sb[:, j * 2 * E:j * 2 * E + E],
                        x2T[j][:], j == 0, j == DT - 1)
                nn_ps = misc_ps((E, TL), "nn")
                for j in range(DT):
                    _mm(nc, nn_ps[:],
                        wrn_sb[:, j * 2 * E + E:(j + 1) * 2 * E],
                        x2T[j][:], j == 0, j == DT - 1)
                logits = spool.tile([E, TL], F32, tag="logits", name="logits",
                                    bufs=1)
                nc.vector.tensor_scalar(logits[:], rt_ps[:],
                                        brb_sb[:, 0:1], None, ALU.add)
                spe = spool.tile([E, TL], BF16, tag="softpe", name="softpe",
                                 bufs=1)
                nc.scalar.activation(spe[:], nn_ps[:], AF.Exp,
                                     bias=brb_sb[:, 1:2])
                spe1 = spool.tile([E, TL], BF16, tag="softpe1",
                                  name="softpe1", bufs=1)
                nc.vector.tensor_scalar_add(spe1[:], spe[:], 1.0)
                sp = spool.tile([E, TL], BF16, tag="softp", name="softp",
                                bufs=1)
                nc.scalar.activation(sp[:], spe1[:], AF.Ln)
                noiseT_sb = spool.tile([E, TL], BF16, tag="noiseTs",
                                       name="noiseTs", bufs=1)
                nc.sync.dma_start(noiseT_sb[:], noiseT_d[:])
                nsp = spool.tile([E, TL], BF16, tag="nsp", name="nsp", bufs=1)
                nc.vector.tensor_tensor(nsp[:], noiseT_sb[:], sp[:], ALU.mult)
                noisy_cm = spool.tile([E, TL], F32, tag="noisycm", name="noisycm",
                                      bufs=1)
                nc.vector.tensor_tensor(noisy_cm[:], nsp[:], logits[:], ALU.add)

                # ---------- top-2 gates (TM); single batched exp ----------
                noisy8 = ppool.tile([128, 8 * NT], F32, tag="noisy8",
                                    name="noisy8")
                nc.vector.memset(noisy8[:], -1e30)
                m8 = ppool.tile([128, 8 * NT], F32, tag="m8", name="m8")
                gate = ppool.tile([128, E * NT], F32, tag="gate", name="gate")
                mask = ppool.tile([128, E * NT], F32, tag="mask", name="mask")
                geT = ppool.tile([E, TL], F32R, tag="geT", name="geT")
                cnt_sb = ppool.tile([1, NT * E], F32, tag="cntsb", name="cntsb")
                for j in range(NT):
                    tr_ps = misc_ps((128, E), "ntr")
                    nc.tensor.matmul(tr_ps[:],
                                     noisy_cm[:, j * 128:(j + 1) * 128],
                                     eye_sb[0:E, 0:E], is_transpose=True,
                                     start=True, stop=True)
                    nc.vector.tensor_copy(noisy8[:, 8 * j:8 * j + E], tr_ps[:])
                for j in range(NT):
                    nc.vector.max(m8[:, 8 * j:8 * j + 8],
                                  noisy8[:, 8 * j:8 * j + 8])
                m8v = m8[:].rearrange("p (j c) -> p j c", c=8)
                dall = spool.tile([128, NT], F32, tag="dall", name="dall",
                                  bufs=1)
                nc.vector.tensor_tensor(dall[:], m8v[:, :, 1], m8v[:, :, 0],
                                        ALU.subtract)
                # sigma(d) = 1/(1+exp(-d)) -- uses the exp table already
                # resident from attention (tanh would force a table swap
                # between the gelu loads)
                emd = spool.tile([128, NT], F32, tag="th", name="emd", bufs=1)
                nc.scalar.activation(emd[:], dall[:], AF.Exp, scale=-1.0)
                ope = spool.tile([128, NT], F32, tag="ope", name="ope", bufs=1)
                nc.vector.tensor_scalar_add(ope[:], emd[:], 1.0)
                spos = spool.tile([128, NT], F32, tag="spos", name="spos",
                                  bufs=1)
                nc.vector.reciprocal(spos[:], ope[:])
                sneg = spool.tile([128, NT], F32, tag="sneg", name="sneg",
                                  bufs=1)
                nc.vector.tensor_scalar(sneg[:], spos[:], -1.0, 1.0, ALU.mult,
                                        ALU.add)
                for j in range(NT):
                    nm = noisy8[:, 8 * j:8 * j + E]
                    v1 = m8[:, 8 * j:8 * j + 1]
                    v2 = m8[:, 8 * j + 1:8 * j + 2]
                    oh1 = spool.tile([128, E], F32, tag="oh1", name="oh1")
                    nc.vector.tensor_scalar(oh1[:], nm, v1, None, ALU.is_ge)
                    msk = mask[:, E * j:E * (j + 1)]
                    nc.vector.tensor_scalar(msk, nm, v2, None, ALU.is_ge)
                    oh2 = spool.tile([128, E], F32, tag="oh2", name="oh2")
                    nc.vector.tensor_tensor(oh2[:], msk, oh1[:], ALU.subtract)
                    g1 = spool.tile([128, E], F32, tag="gnum", name="g1")
                    nc.vector.tensor_scalar(g1[:], oh1[:],
                                            sneg[:, j:j + 1], None, ALU.mult)
                    g2 = spool.tile([128, E], F32, tag="gnum2", name="g2")
                    nc.vector.tensor_scalar(g2[:], oh2[:],
                                            spos[:, j:j + 1], None, ALU.mult)
                    nc.vector.tensor_tensor(gate[:, E * j:E * (j + 1)],
                                            g1[:], g2[:], ALU.add)
                    cps = misc_ps((1, E), "cnt")
                    _mm(nc, cps[:], ones_sb[:, 0:1], msk, True, True, F32)
                    nc.vector.tensor_copy(cnt_sb[0:1, E * j:E * (j + 1)], cps[:])

                # total counts -> all-gather
                tot = spool.tile([1, E], F32, tag="cnttot", name="cnttot",
                                 bufs=1)
                nc.vector.tensor_tensor(tot[:], cnt_sb[0:1, 0:E],
                                        cnt_sb[0:1, E:2 * E], ALU.add)
                nc.vector.tensor_tensor(tot[:], tot[:], cnt_sb[0:1, 2 * E:3 * E],
                                        ALU.add)
                nc.vector.tensor_tensor(tot[:], tot[:], cnt_sb[0:1, 3 * E:4 * E],
                                        ALU.add)
                nc.sync.dma_start(cnt_in[:], tot[:])
                nc.gpsimd.collective_compute(
                    "AllGather", ALU.bypass, replica_groups=rg_all,
                    ins=[cnt_in[:].opt()], outs=[cnt_out[:].opt()])

            # ---------- MLP hidden + MoE hidden (overlaps counts AG) ------
            Hm_sb = []
            for m in range(MLPH // 128):
                slab = wpool.tile([128, DT * 128], BF16, tag="qkslab",
                                  name="m1slab")
                nc.sync.dma_start(
                    slab[:], wmlp1_d[:, m * 1024:(m + 1) * 1024])
                ps = big_ps("hm")
                for kk in range(DT):
                    _mm(nc, ps[:], slab[:, kk * 128:(kk + 1) * 128],
                        x2T[kk][:], kk == 0, kk == DT - 1)
                hm = ppool.tile([128, TL], BF16, tag=f"hm{m}", name=f"hm{m}")
                nc.scalar.activation(hm[:], ps[:], AF.Gelu,
                                     bias=bmlp1_sb[:, m:m + 1])
                Hm_sb.append(hm)
            Hmoe = []
            for e in range(E):
                for hmi in range(MOEH // 128):
                    me = 2 * e + hmi
                    slab = wpool.tile([128, DT * 128], BF16, tag="qkslab",
                                      name="e1slab")
                    nc.sync.dma_start(
                        slab[:], we1_d[:, me * 1024:(me + 1) * 1024])
                    ps = big_ps("hmoe")
                    for kk in range(DT):
                        _mm(nc, ps[:], slab[:, kk * 128:(kk + 1) * 128],
                            x2T[kk][:], kk == 0, kk == DT - 1)
                    hs = ppool.tile([128, TL], BF16, tag=f"hmoe{me}",
                                    name=f"hmoe{me}")
                    nc.scalar.activation(
                        hs[:], ps[:], AF.Gelu,
                        bias=be1_sb[:, me:me + 1])
                    Hmoe.append(hs)

            # ---------- ranks / keep / gate_eff ----------
            cntg = spool.tile([NC, E], F32, tag="cntg", name="cntg", bufs=1)
            nc.sync.dma_start(cntg[:], cnt_out[:])
            off_ps = misc_ps((1, E), "off")
            _mm(nc, off_ps[:], wpfx_sb[:], cntg[:], True, True, F32)
            car = spool.tile([1, E * NT], F32, tag="car", name="car", bufs=1)
            nc.vector.tensor_copy(car[:, 0:E], off_ps[:])
            for j in range(1, NT):
                nc.vector.tensor_tensor(car[:, E * j:E * (j + 1)],
                                        car[:, E * (j - 1):E * j],
                                        cnt_sb[0:1, E * (j - 1):E * j],
                                        ALU.add)
            ge_tm = ppool.tile([128, E * NT], F32, tag="getm", name="getm")
            for j in range(NT):
                rk_ps = misc_ps((128, E), "rank")
                _mm(nc, rk_ps[:], utri_sb[:],
                    mask[:, E * j:E * (j + 1)], True, False, F32)
                _mm(nc, rk_ps[:], ones_sb[0:1, :],
                    car[:, E * j:E * (j + 1)], False, True, F32)
                keep = spool.tile([128, E], F32, tag="keep", name="keep")
                nc.vector.tensor_scalar(keep[:], rk_ps[:], float(CAP), None,
                                        ALU.is_lt)
                nc.vector.tensor_tensor(ge_tm[:, E * j:E * (j + 1)],
                                        gate[:, E * j:E * (j + 1)],
                                        keep[:], ALU.mult)
            for j in range(NT):
                tr_ps = misc_ps((E, 128), "getr")
                nc.tensor.matmul(tr_ps[:], ge_tm[:, E * j:E * (j + 1)],
                                 eye_sb[:, :], is_transpose=True,
                                 start=True, stop=True)
                nc.vector.tensor_copy(geT[:, j * 128:(j + 1) * 128], tr_ps[:])

            # gate the MoE hidden
            Hg = []
            for e in range(E):
                bc_ps = misc_ps((128, TL), "gbc")
                _mm(nc, bc_ps[:], gsel_sb[:, e * 128:(e + 1) * 128],
                    geT[:], True, True)
                bc_sb = spool.tile([128, TL], BF16, tag="gbcsb", name="gbcsb",
                                   bufs=2)
                nc.vector.tensor_copy(bc_sb[:], bc_ps[:])
                for hmi in range(MOEH // 128):
                    hg = ppool.tile([128, TL], BF16, tag=f"hg{2*e+hmi}",
                                    name=f"hg{2*e+hmi}")
                    nc.vector.tensor_tensor(hg[:], Hmoe[2 * e + hmi][:],
                                            bc_sb[:], ALU.mult)
                    Hg.append(hg)

            # ---------- output GEMM: moe + be2 + mlp, fused accum ----------
            for m in range(DT):
                slab = wpool.tile([128, 40 * 128], BF16, tag="outslab",
                                  name="outslab")
                nc.sync.dma_start(
                    slab[:], wout_d[:, m * 5120:(m + 1) * 5120])
                ps = big_ps("out")
                for kk in range(MLPH // 128):   # gate-independent part first
                    _mm(nc, ps[:],
                        slab[:, (8 + kk) * 128:(9 + kk) * 128],
                        Hm_sb[kk][:], kk == 0, False)
                for i8 in range(8):          # we2 tiles (e, hmi)
                    _mm(nc, ps[:], slab[:, i8 * 128:(i8 + 1) * 128],
                        Hg[i8][:], False, False)
                _mm(nc, ps[:], be2_sb[:, m * 128:(m + 1) * 128],
                    geT[:], False, True)
                o = spool.tile([128, TL], F32, tag="outsb", name="outsb",
                               bufs=2)
                nc.vector.scalar_tensor_tensor(
                    o[:], ps[:], bmlp2_sb[:, m:m + 1], xres[m][:],
                    ALU.add, ALU.add)
                nc.sync.dma_start(out_d[m * 128:(m + 1) * 128, :], o[:])

    nc.compile()
    return nc


def _tile_lhst(w, n_k, n_m):
    # w: [n_k*128, n_m*128] -> [128, n_m, n_k, 128] -> [128, n_m*n_k*128]
    kdim, mdim = w.shape
    return np.ascontiguousarray(
        w.reshape(n_k, 128, n_m, 128).transpose(1, 2, 0, 3)
        .reshape(128, n_m * n_k * 128))


def _cols(a, n):
    # [n*128] -> [128, n] with column j = chunk j
    return np.ascontiguousarray(
        np.asarray(a, np.float32).reshape(n, 128).T)


def _prep_inputs(inputs):
    f32 = lambda a: np.ascontiguousarray(np.asarray(a, np.float32))
    bf = lambda a: np.ascontiguousarray(
        np.asarray(a, np.float32).astype(ml_dtypes.bfloat16))
    f8 = lambda a: np.ascontiguousarray(
        np.asarray(a, np.float32).astype(ml_dtypes.float8_e4m3))
    x = f32(inputs["x"]).reshape(T, D)
    noise = f32(inputs["noise"]).reshape(T, E)
    w_qkv = np.asarray(inputs["w_qkv"], np.float32)
    wqkT = w_qkv[:2 * D].T                       # [D, 2048]
    wvT = w_qkv[2 * D:].T                        # [D, D]
    wprojT = np.asarray(inputs["w_proj"], np.float32).T
    we1 = np.asarray(inputs["we1"], np.float32)  # [E, D, MOEH]
    we2 = np.asarray(inputs["we2"], np.float32)  # [E, MOEH, D]
    wmlp1 = np.asarray(inputs["w_mlp1"], np.float32)   # [D, MLPH]
    wmlp2 = np.asarray(inputs["w_mlp2"], np.float32)   # [MLPH, D]

    # we1 slabs: m-index = e*2+hmi over [D, 256] each
    we1_flat = np.concatenate([we1[e] for e in range(E)], 1)  # [D, E*MOEH]
    # wout: per m, 8 we2 tiles (e,hmi) then 32 wmlp2 tiles
    we2_l = we2.reshape(E, 2, 128, DT, 128).transpose(2, 3, 0, 1, 4) \
        .reshape(128, DT, 8, 128)
    wm2_l = wmlp2.reshape(32, 128, DT, 128).transpose(1, 2, 0, 3)
    wout = np.concatenate([we2_l, wm2_l], 2).reshape(128, DT * 40 * 128)

    shared = dict(
        wqk_l=f8(_tile_lhst(wqkT, DT, 16)),
        wv_l=f8(np.ascontiguousarray(
            wvT.reshape(DT, 128, D).transpose(1, 0, 2).reshape(128, DT * D))),
        wproj_l=f8(_tile_lhst(wprojT, DT, DT)),
        wmlp1_l=bf(_tile_lhst(wmlp1, DT, 32)),
        we1_l=bf(_tile_lhst(we1_flat, DT, 8)),
        wout_l=bf(wout),
        wrn=bf(np.concatenate([f32(inputs["w_route"]),
                               f32(inputs["w_noise"])], 1)
               .reshape(DT, 128, 2 * E).transpose(1, 0, 2)
               .reshape(128, DT * 2 * E)),
        be2=f32(inputs["be2"]),
        ln1g=_cols(inputs["ln1_g"], DT),
        ln1b=_cols(inputs["ln1_b"], DT),
        ln2g=_cols(inputs["ln2_g"], DT),
        ln2b=_cols(inputs["ln2_b"], DT),
        bproj=_cols(inputs["b_proj"], DT),
        be1=_cols(inputs["be1"], E * MOEH // 128),
        bmlp1=_cols(inputs["b_mlp1"], MLPH // 128),
        bmlp2=_cols(inputs["b_mlp2"], DT),
        brb=np.ascontiguousarray(np.stack(
            [f32(inputs["b_route"]), f32(inputs["b_noise"])], 1)),
        ones128=np.ones((128, 128), np.float32),
        eye128=np.eye(128, dtype=np.float32),
        utri128=np.triu(np.ones((128, 128), np.float32), 1),
        gsel=np.repeat(np.eye(E, dtype=np.float32), 128, 1),
        gsel8=np.ascontiguousarray((np.arange(8)[:, None] == (
            2 * (np.arange(512) // 128) + (np.arange(512) % 128) // 64
        )[None, :]).astype(np.float32)),
        ones128b=np.ones((128, 1), ml_dtypes.bfloat16),
    )
    in_maps = []
    for c in range(NC):
        m = dict(shared)
        m["xT"] = bf(x[c * TL:(c + 1) * TL].T)
        m["noiseT"] = bf(noise[c * TL:(c + 1) * TL].T)
        m["wpfx"] = (np.arange(NC) < c).astype(np.float32).reshape(NC, 1)
        in_maps.append(m)
    return in_maps


def _run(inputs, trace=False):
    if "nc" not in _cache:
        _cache["nc"] = _build()
    nc = _cache["nc"]
    in_maps = _prep_inputs(inputs)
    res = run_bass_kernel_spmd(nc, in_maps, core_ids=list(range(NC)),
                               trace=trace)
    _cache["last_res"] = res
    shards = [res.results[c]["out"] for c in range(NC)]   # each [D, TL]
    out = np.concatenate([np.asarray(s, np.float32).T for s in shards],
                         0).reshape(B, N, D)
    return out.astype(np.float32), res.exec_time_ns


def kernel(**inputs):
    out, _ = _run(inputs, trace=False)
    return out



# revision 23
# speedup vs baseline: 1.5123x; 1.0440x over previous
"""Trainium2 Bass kernel for nn_BlockMoEAdapters (8 NeuronCores, SPMD).

Sharding: tokens (B*N = 4096) split contiguously across 8 cores (512 each).
Cores 0-3 hold batch 0, cores 4-7 batch 1. Attention K/V are all-gathered
(bf16, per half: k then v, launched as soon as each half's GEMM finishes so
the gather hides behind the q GEMM) within each 4-core batch group; MoE
capacity ranks use a tiny 8-core all-gather of per-core expert counts.

Layout: channel-major ([channels, tokens]) on-device for all GEMMs; LayerNorm
stats via ones-matmul partition reductions, rsqrt via ln/exp on the scalar
engine (keeps a single natural_log_exp activation table until the gelu
phase); softmax in [keys, tokens] orientation with the denominator from a
ones-column appended to V; denominator rows are gathered into [8, TL] tiles
(DVE copy + sbuf-to-sbuf DMA), reciprocated in one batched DVE op per group
of 4 pairs, and broadcast back per pair with a selector matmul; attention is
software-pipelined one beat ahead with the two heads of a pair issued to
disjoint PE row groups so their K=64 score matmuls run concurrently. Weights
host-retiled into per-output-slab layouts; output re-transposed on host.
"""
import sys

for _p in ('/opt/trn_rl_repo',):
    if _p not in sys.path:
        sys.path.append(_p)

import ml_dtypes
import numpy as np

import concourse.bass as bass
import concourse.mybir as mybir
import concourse.tile as tile
from concourse import bacc
from concourse.bass_utils import run_bass_kernel_spmd

F32 = mybir.dt.float32
F32R = mybir.dt.float32r
BF16 = mybir.dt.bfloat16
FP8 = mybir.dt.float8e4
I8 = mybir.dt.int8
AF = mybir.ActivationFunctionType
ALU = mybir.AluOpType
DR = mybir.MatmulPerfMode.DoubleRow
# Schraudolph exp in fp8-e4m3 bit domain: bits = round(z/ln2*8 + 7*8 + c)
# for weight = exp(0.125*score); z = 0.125*score folded into the scale.
SCH_A = 0.125 * 8.0 / np.log(2.0)
SCH_B = 56.0 - 0.344 + 0.5   # +0.5 centers the truncating float->int cast

B, N, D = 2, 2048, 1024
H, HD = 16, 64
E, TOPK = 4, 2
MOEH, MLPH = 256, 4096
T = B * N
NC = 8
TL = T // NC          # 512 tokens per core
NT = TL // 128        # 4 token tiles
DT = D // 128         # 8 channel tiles
CAP = int(T * TOPK / E * 1.0)   # 2048
GRP = 4               # cores per kv-gather group
EPS = 1e-5

_cache = {}


def _mm(nc, out, lhsT, rhs, start, stop, dt=None):
    if dt is not None:
        lhsT, rhs = lhsT.bitcast(dt), rhs.bitcast(dt)
    nc.tensor.matmul(out, lhsT, rhs, start=start, stop=stop)


def _build():
    nc = bacc.Bacc("TRN2", target_bir_lowering=False, debug=False,
                   num_devices=NC)

    def din(name, shape, dt=F32):
        return nc.dram_tensor(name, list(shape), dt, kind="ExternalInput")

    xT_d = din("xT", (D, TL), BF16)
    noiseT_d = din("noiseT", (E, TL), BF16)
    # host-retiled weight slabs (see _prep_inputs for layouts)
    wqk_d = din("wqk_l", (128, 16 * DT * 128), FP8)
    wv_d = din("wv_l", (128, DT * 1024), FP8)
    wproj_d = din("wproj_l", (128, DT * DT * 128), FP8)
    wmlp1_d = din("wmlp1_l", (128, 32 * DT * 128), BF16)
    we1_d = din("we1_l", (128, 8 * DT * 128), BF16)
    wout_d = din("wout_l", (128, DT * 40 * 128), BF16)
    wrn_d = din("wrn", (128, DT * 2 * E), BF16)   # w_route || w_noise tiles
    be2_d = din("be2", (E, D))
    # column-tiled small constants: [128, n] with col j = chunk j
    ln1g_d = din("ln1g", (128, DT))
    ln1b_d = din("ln1b", (128, DT))
    ln2g_d = din("ln2g", (128, DT))
    ln2b_d = din("ln2b", (128, DT))
    bproj_d = din("bproj", (128, DT))
    be1_d = din("be1", (128, E * MOEH // 128))
    bmlp1_d = din("bmlp1", (128, MLPH // 128))
    bmlp2_d = din("bmlp2", (128, DT))
    brb_d = din("brb", (E, 2))               # broute | bnoise columns
    ones_d = din("ones128", (128, 128))
    onesb_d = din("ones128b", (128, 1), BF16)
    eye_d = din("eye128", (128, 128))
    utri_d = din("utri128", (128, 128))      # U[s,t] = 1 if s < t
    gsel_d = din("gsel", (E, E * 128))       # gsel[k, e*128+p] = (k == e)
    gsel8_d = din("gsel8", (8, 4 * 128))     # gsel8[h, q*128+c] = (h==2q+c//64)
    wpfx_d = din("wpfx", (NC, 1))            # per-core: 1 for j < core_id

    out_d = nc.dram_tensor("out", [D, TL], F32, kind="ExternalOutput")

    rg_kv = [[0, 1, 2, 3], [4, 5, 6, 7]]
    rg_all = [list(range(NC))]

    with tile.TileContext(nc) as tc:
        with (
            tc.tile_pool(name="dram", bufs=1, space="DRAM") as dpool,
            tc.tile_pool(name="consts", bufs=1) as cpool,
            tc.tile_pool(name="persist", bufs=1) as ppool,
            # PSUM: 's2' 3x[128,1024] = 6 banks, 'ao' 2x[128,512] = 2 banks
            tc.tile_pool(name="ps_s2", bufs=3, space="PSUM") as ps_s2,
            tc.tile_pool(name="ps_ao", bufs=2, space="PSUM") as ps_ao,
            tc.tile_pool(name="wslab", bufs=2) as wpool,
            tc.tile_pool(name="scratch", bufs=2) as spool,
        ):
            def big_ps(name):
                # [128, TL] psum tile in an 's2'-class slot
                return ps_s2.tile([128, TL], F32, tag="s2", name=name)

            def misc_ps(shape, name):
                return ps_ao.tile(list(shape), F32, tag="ao", name=name)

            # ---------- collective bounce buffers (k / v per half) --------
            # kept under 1 MB per message so the collective stays on the
            # fast mesh algorithm (>=1 MB switches to ring, ~2.5x slower);
            # v travels in fp8 (it feeds the fp8 DoubleRow AV matmul anyway)
            KSZ = (D // 2) * TL
            VSZ = 128 * 4 * 640
            KVSZ = KSZ + VSZ          # 576 KB < 1 MB keeps the mesh algo
            kv_in = [dpool.tile([1, KVSZ], FP8, name=f"kv_in{h_}")
                     for h_ in range(2)]
            kv_out = [dpool.tile([GRP, KVSZ], FP8, name=f"kv_out{h_}")
                      for h_ in range(2)]
            cnt_in = dpool.tile([1, E], F32, name="cnt_in")
            cnt_out = dpool.tile([NC, E], F32, name="cnt_out")

            # ---------- constants (gpsimd queue, one DMA each) ----------
            def load_const(dram, shape, dt=F32, tag=None):
                tag = tag or dram.name
                t = cpool.tile(list(shape), dt, tag=tag, name=tag)
                nc.gpsimd.dma_start(t[:], dram[:])
                return t

            ones_sb = load_const(ones_d, (128, 128))
            onesr_sb = load_const(ones_d, (128, 128), F32R, tag="ones128r")
            onesb_sb = load_const(onesb_d, (128, 1), BF16)
            gsel_sb = load_const(gsel_d, (E, E * 128), F32R)
            gsel8_sb = load_const(gsel8_d, (8, 4 * 128), F32R)
            eye_sb = load_const(eye_d, (128, 128))
            utri_sb = load_const(utri_d, (128, 128))
            wpfx_sb = load_const(wpfx_d, (NC, 1))
            brb_sb = load_const(brb_d, (E, 2))
            be2_sb = load_const(be2_d, (E, D), F32R)
            ln1g_sb = load_const(ln1g_d, (128, DT))
            ln1b_sb = load_const(ln1b_d, (128, DT))
            ln2g_sb = load_const(ln2g_d, (128, DT))
            ln2b_sb = load_const(ln2b_d, (128, DT))
            bproj_sb = load_const(bproj_d, (128, DT))
            be1_sb = load_const(be1_d, (128, E * MOEH // 128))
            bmlp1_sb = load_const(bmlp1_d, (128, MLPH // 128))
            bmlp2_sb = load_const(bmlp2_d, (128, DT))
            wrn_sb = load_const(wrn_d, (128, DT * 2 * E), BF16)

            # ---------- load x (CM, bf16 for GEMM-side) ----------
            xT_sb = []
            for j in range(DT):
                t = ppool.tile([128, TL], BF16, tag=f"xT{j}", name=f"xT{j}")
                nc.sync.dma_start(t[:], xT_d[j * 128:(j + 1) * 128, :])
                xT_sb.append(t)

            epsc = cpool.tile([1, 1], F32, tag="epsc", name="epsc")
            nc.vector.memset(epsc[:], EPS)

            # ---------- LayerNorm in CM; bf16 in/out tiles ----------
            def layernorm_cm(xtiles, g_sb, b_sb, outtag, out_aps=None):
                ones_col = onesb_sb[:, 0:1]
                musum = misc_ps((1, TL), "musum")
                sqsum = misc_ps((1, TL), "sqsum")
                for j in range(DT):
                    sq = spool.tile([128, TL], BF16, tag="lnsq", name="lnsq",
                                    bufs=2)
                    nc.vector.tensor_tensor(sq[:], xtiles[j][:], xtiles[j][:],
                                            ALU.mult)
                    _mm(nc, musum[:], ones_col, xtiles[j][:],
                        j == 0, j == DT - 1)
                    _mm(nc, sqsum[:], ones_col, sq[:],
                        j == 0, j == DT - 1)
                mu = spool.tile([1, TL], F32R, tag="lnmu", name="lnmu", bufs=1)
                nc.vector.tensor_scalar_mul(mu[:], musum[:], 1.0 / D)
                msq = spool.tile([1, TL], F32, tag="lnscr", name="lnmsq",
                                 bufs=2)
                nc.vector.tensor_tensor(msq[:], mu[:].bitcast(F32),
                                        mu[:].bitcast(F32), ALU.mult)
                var = spool.tile([1, TL], F32, tag="lnscr", name="lnvar",
                                 bufs=2)
                nc.vector.scalar_tensor_tensor(var[:], sqsum[:], 1.0 / D,
                                               msq[:], ALU.mult, ALU.subtract)
                # rsqrt(var+eps) = exp(-0.5*ln(var+eps)) (stays in nle set)
                lnv = spool.tile([1, TL], F32, tag="lnscr", name="lnlnv",
                                 bufs=2)
                nc.scalar.activation(lnv[:], var[:], AF.Ln, bias=epsc[:])
                rsig = spool.tile([1, TL], F32R, tag="lnrsig", name="lnrsig",
                                  bufs=1)
                nc.scalar.activation(rsig[:], lnv[:], AF.Exp, scale=-0.5)
                mub_ps = misc_ps((128, TL), "mub")
                _mm(nc, mub_ps[:], onesr_sb[0:1, :], mu[:], True, True)
                rsb_ps = misc_ps((128, TL), "rsb")
                _mm(nc, rsb_ps[:], onesr_sb[0:1, :], rsig[:], True, True)
                outs = []
                for j in range(DT):
                    t1 = spool.tile([128, TL], F32, tag="lnt1", name="lnt1",
                                    bufs=2)
                    nc.vector.tensor_tensor(t1[:], xtiles[j][:], mub_ps[:],
                                            ALU.subtract)
                    t2 = spool.tile([128, TL], F32, tag="lnt2", name="lnt2",
                                    bufs=2)
                    nc.vector.tensor_tensor(t2[:], t1[:], rsb_ps[:], ALU.mult)
                    if out_aps is None:
                        o = ppool.tile([128, TL], BF16, tag=f"{outtag}{j}",
                                       name=f"{outtag}{j}")
                        oa = o[:]
                    else:
                        o, oa = None, out_aps[j]
                    nc.scalar.activation(oa, t2[:], AF.Identity,
                                         bias=b_sb[:, j:j + 1],
                                         scale=g_sb[:, j:j + 1])
                    outs.append(o)
                return outs

            qT_sb = [ppool.tile([128, TL], FP8, tag=f"qT{m}", name=f"qT{m}")
                     for m in range(DT)]

            x1a = ppool.tile([128, DT * TL], FP8, tag="x1all",
                             name="x1all")
            with tc.tile_pool(name="st1", bufs=2) as s1pool:
                layernorm_cm(xT_sb, ln1g_sb, ln1b_sb, "x1T",
                             out_aps=[x1a[:, j * TL:(j + 1) * TL]
                                      for j in range(DT)])
                x1f = x1a[:].rearrange("p (k x) -> p k x", x=TL)

                def qk_gemm(m):
                    # m 0-7: q slabs; 8-15: k slabs (fp8 DoubleRow over
                    # adjacent contraction-tile pairs)
                    slab = wpool.tile([128, DT * 128], FP8, tag="qkslab8",
                                      name="qkslab")
                    nc.sync.dma_start(
                        slab[:], wqk_d[:, m * 1024:(m + 1) * 1024])
                    ps = big_ps("qk")
                    sl = slab[:].rearrange("p (k c) -> p k c", c=128)
                    for kk in range(DT):
                        nc.tensor.matmul(ps[:], sl[:, kk, :], x1f[:, kk, :],
                                         start=kk == 0, stop=kk == DT - 1)
                    if m < DT:
                        nc.vector.tensor_copy(qT_sb[m][:], ps[:])
                    else:
                        ksb = s1pool.tile([128, TL], FP8, tag="kevac",
                                          name="kevac", bufs=2)
                        nc.vector.tensor_copy(ksb[:], ps[:])
                        mk = m - DT
                        nc.gpsimd.dma_start(
                            kv_in[mk // 4][0:1, (mk % 4) * 128 * TL:
                                           (mk % 4 + 1) * 128 * TL], ksb[:])

                def v_gemm(nn):
                    # v output channels nn*512 .. (nn+1)*512, TM layout + pad
                    wv_slabs = []
                    for j in range(DT // 2):
                        s = s1pool.tile([128, 2 * 512], FP8, tag=f"wv{j}",
                                        name=f"wv{j}", bufs=1)
                        for i in range(2):
                            kk = 2 * j + i
                            nc.sync.dma_start(
                                s[:, i * 512:(i + 1) * 512],
                                wv_d[:, kk * 1024 + nn * 512:
                                     kk * 1024 + (nn + 1) * 512])
                        wv_slabs.append(s)
                    for mt in range(NT):
                        ps = big_ps("vps")
                        for kk in range(DT):
                            nc.tensor.matmul(
                                ps[:],
                                x1f[:, kk, mt * 128:(mt + 1) * 128],
                                wv_slabs[kk // 2][:, (kk % 2) * 512:
                                                  (kk % 2 + 1) * 512],
                                start=kk == 0, stop=kk == DT - 1)
                        vp = s1pool.tile([128, 640], FP8, tag="vpad",
                                         name="vpad", bufs=2)
                        nc.vector.memset(vp[:], 1.0)
                        dst = vp[:].rearrange("p (h c) -> p h c", c=80)
                        nc.vector.tensor_copy(
                            dst[:, :, 0:64],
                            ps[:].rearrange("p (h c) -> p h c", c=64))
                        nc.gpsimd.dma_start(
                            kv_in[nn][0:1, KSZ:KSZ + VSZ].rearrange(
                                "a (p q c) -> a p q c", q=4,
                                c=640)[:, :, :, mt * 160:(mt + 1) * 160],
                            vp[:].rearrange("p (q c) -> p q c", c=160))

                def ag(buf_in, buf_out):
                    nc.gpsimd.collective_compute(
                        "AllGather", ALU.bypass, replica_groups=rg_kv,
                        ins=[buf_in[:].opt()], outs=[buf_out[:].opt()])

                # all k/v first so the serial CC chain starts ASAP; q last
                # (q is only needed when attention starts)
                for m in (8, 9, 10, 11):
                    qk_gemm(m)
                v_gemm(0)
                ag(kv_in[0], kv_out[0])
                for m in (12, 13, 14, 15):
                    qk_gemm(m)
                v_gemm(1)
                ag(kv_in[1], kv_out[1])
                for m in range(DT):       # q
                    qk_gemm(m)

            # ---------- attention (pipelined beats, row-packed heads) -----
            # ao reuses the x1 slot (dead once the qkv GEMMs finish);
            # fp8 so proj can run as DoubleRow
            ao_a = ppool.tile([128, DT * TL], FP8, tag="x1all",
                              name="ao_all")   # pair p cols: rows 0:64 head 2p
            with (
                tc.tile_pool(name="attn", bufs=2) as apool,
                tc.tile_pool(name="vsb", bufs=2) as vpool,
                tc.tile_pool(name="ssb", bufs=4) as spool_s,
            ):
                # denominators for pairs 0-3 / 4-7 (partition = head mod 8)
                den = [ppool.tile([8, TL], F32, tag=f"den{g}",
                                  name=f"den{g}") for g in range(2)]

                def den_finish(g):
                    # reciprocal + per-pair broadcast + normalize
                    r8 = spool_s.tile([8, TL], F32R, tag="r8", name="r8",
                                      bufs=1)
                    with nc.allow_low_precision(reason="f32r recip for bcast"):
                        nc.vector.reciprocal(r8[:], den[g][:])
                    for q in range(4):
                        p = 4 * g + q
                        bcp = misc_ps((128, TL), "dbc")
                        _mm(nc, bcp[:], gsel8_sb[:, q * 128:(q + 1) * 128],
                            r8[:], True, True)
                        nc.vector.tensor_tensor(
                            ao_a[:, p * TL:(p + 1) * TL],
                            ao_a[:, p * TL:(p + 1) * TL], bcp[:], ALU.mult)

                for p in range(DT):              # head pair
                    hf, pq = p // 4, p % 4       # kv half, pair in half
                    kp = []
                    vt = []
                    for r in range(GRP):
                        kt_ = apool.tile([128, TL], FP8, tag=f"kp{r}",
                                         name=f"kp{r}")
                        nc.sync.dma_start(
                            kt_[:],
                            kv_out[hf][r:r + 1,
                                       pq * 128 * TL:(pq + 1) * 128 * TL]
                            .rearrange("a (p c) -> a p c", c=TL))
                        kp.append(kt_)
                        vt_ = vpool.tile([128, 640], FP8, tag=f"vt{r}",
                                         name=f"vt{r}")
                        nc.sync.dma_start(
                            vt_[:],
                            kv_out[hf][r:r + 1, KSZ:KSZ + VSZ].rearrange(
                                "a (p q c) -> a p q c", q=4,
                                c=640)[:, :, pq, :])
                        vt.append(vt_)
                    ao_ps = [ps_ao.tile([65, TL], F32, tag="ao",
                                        name=f"ao{hh}") for hh in range(2)]
                    ssb = {}

                    def scores(beat):
                        s2 = [ps_s2.tile([128, 2 * TL], F32, tag="s2",
                                         name=f"s2_{hh}") for hh in range(2)]
                        # interleave heads so the K=64 matmuls land in
                        # disjoint PE row groups and run concurrently
                        for u in range(2):
                            kt = 2 * beat + u
                            r, cc = kt // 4, kt % 4
                            for hh in range(2):
                                po = 64 * hh
                                _mm(nc, s2[hh][:, u * TL:(u + 1) * TL],
                                    kp[r][po:po + 64,
                                          cc * 128:(cc + 1) * 128],
                                    qT_sb[p][po:po + 64, :], True, True)
                        for hh in range(2):
                            # softmax exp -> fp8 weights; head 0 exact on the
                            # scalar engine, head 1 via the Schraudolph
                            # exp-in-bit-domain trick on the (idle) DVE so the
                            # two run concurrently
                            s_sb = spool_s.tile([128, 2 * TL], FP8,
                                                tag="ssb", name="ssb")
                            if hh == 0:
                                nc.scalar.activation(s_sb[:], s2[hh][:],
                                                     AF.Exp, scale=0.125)
                            else:
                                nc.vector.tensor_scalar(
                                    s_sb[:].bitcast(I8), s2[hh][:],
                                    SCH_A, SCH_B, ALU.mult, ALU.add)
                            ssb[(beat, hh)] = s_sb

                    def avs(beat):
                        # fp8 DoubleRow: one matmul per head folds both key
                        # tiles (consecutive cc on the same partitions)
                        cc = (2 * beat) % 4
                        r = (2 * beat) // 4
                        for hh in range(2):
                            s_sb = ssb.pop((beat, hh))
                            sv = s_sb[:].rearrange("p (u x) -> p u x", x=TL)
                            vv = vt[r][:].rearrange(
                                "p (c x) -> p c x",
                                x=160)[:, cc:cc + 2, 80 * hh:80 * hh + 65]
                            nc.tensor.matmul(ao_ps[hh][:], vv, sv,
                                             start=beat == 0, stop=beat == 7,
                                             perf_mode=DR)

                    scores(0)
                    for beat in range(1, 8):
                        scores(beat)
                        avs(beat - 1)
                    avs(7)

                    # evacuate unnormalized ao + stash denominator rows
                    for hh in range(2):
                        po = 64 * hh
                        nc.scalar.activation(
                            ao_a[po:po + 64, p * TL:(p + 1) * TL],
                            ao_ps[hh][0:64, :], AF.Copy)
                        dcp = spool_s.tile([1, TL], F32, tag="dcp",
                                           name="dcp", bufs=2)
                        nc.vector.tensor_copy(dcp[:], ao_ps[hh][64:65, :])
                        h8 = 2 * pq + hh
                        nc.sync.dma_start(den[hf][h8:h8 + 1, :], dcp[:])
                    if p == 3:
                        den_finish(0)
                den_finish(1)

                # ---------- proj + residual ----------
                xres = []
                xres_bf = []
                aof = ao_a[:].rearrange("p (k x) -> p k x", x=TL)
                for m in range(DT):
                    slab = wpool.tile([128, DT * 128], FP8, tag="qkslab8",
                                      name="projslab")
                    nc.sync.dma_start(
                        slab[:], wproj_d[:, m * 1024:(m + 1) * 1024])
                    ps = big_ps("proj")
                    sl = slab[:].rearrange("p (k c) -> p k c", c=128)
                    for kk in range(DT):
                        nc.tensor.matmul(ps[:], sl[:, kk, :], aof[:, kk, :],
                                         start=kk == 0, stop=kk == DT - 1)
                    xr = ppool.tile([128, TL], F32, tag=f"xres{m}",
                                    name=f"xres{m}")
                    nc.vector.scalar_tensor_tensor(
                        xr[:], ps[:], bproj_sb[:, m:m + 1], xT_sb[m][:],
                        ALU.add, ALU.add)
                    xb = ppool.tile([128, TL], BF16, tag=f"xresb{m}",
                                    name=f"xresb{m}")
                    nc.vector.tensor_copy(xb[:], xr[:])
                    xres.append(xr)
                    xres_bf.append(xb)

            # ---------- LN2 (x2T reuses the qT slots, dead after attn) ----
            x2T = layernorm_cm(xres_bf, ln2g_sb, ln2b_sb, "x2T")

            # router + gates scheduled at high priority so their
            # exp ops land before the gelu table switch
            with tc.high_priority():
                # ---------- router (route | noise fused GEMM) ----------
                rt_ps = misc_ps((E, TL), "rt")
                for j in range(DT):
                    _mm(nc, rt_ps[:],
                        wrn_sb[:, j * 2 * E:j * 2 * E + E],
                        x2T[j][:], j == 0, j == DT - 1)
                nn_ps = misc_ps((E, TL), "nn")
                for j in range(DT):
                    _mm(nc, nn_ps[:],
                        wrn_sb[:, j * 2 * E + E:(j + 1) * 2 * E],
                        x2T[j][:], j == 0, j == DT - 1)
                logits = spool.tile([E, TL], F32, tag="logits", name="logits",
                                    bufs=1)
                nc.vector.tensor_scalar(logits[:], rt_ps[:],
                                        brb_sb[:, 0:1], None, ALU.add)
                spe = spool.tile([E, TL], BF16, tag="softpe", name="softpe",
                                 bufs=1)
                nc.scalar.activation(spe[:], nn_ps[:], AF.Exp,
                                     bias=brb_sb[:, 1:2])
                spe1 = spool.tile([E, TL], BF16, tag="softpe1",
                                  name="softpe1", bufs=1)
                nc.vector.tensor_scalar_add(spe1[:], spe[:], 1.0)
                sp = spool.tile([E, TL], BF16, tag="softp", name="softp",
                                bufs=1)
                nc.scalar.activation(sp[:], spe1[:], AF.Ln)
                noiseT_sb = spool.tile([E, TL], BF16, tag="noiseTs",
                                       name="noiseTs", bufs=1)
                nc.sync.dma_start(noiseT_sb[:], noiseT_d[:])
                nsp = spool.tile([E, TL], BF16, tag="nsp", name="nsp", bufs=1)
                nc.vector.tensor_tensor(nsp[:], noiseT_sb[:], sp[:], ALU.mult)
                noisy_cm = spool.tile([E, TL], F32, tag="noisycm", name="noisycm",
                                      bufs=1)
                nc.vector.tensor_tensor(noisy_cm[:], nsp[:], logits[:], ALU.add)

                # ---------- top-2 gates (TM); single batched exp ----------
                noisy8 = ppool.tile([128, 8 * NT], F32, tag="noisy8",
                                    name="noisy8")
                nc.vector.memset(noisy8[:], -1e30)
                m8 = ppool.tile([128, 8 * NT], F32, tag="m8", name="m8")
                gate = ppool.tile([128, E * NT], F32, tag="gate", name="gate")
                mask = ppool.tile([128, E * NT], F32, tag="mask", name="mask")
                geT = ppool.tile([E, TL], F32R, tag="geT", name="geT")
                cnt_sb = ppool.tile([1, NT * E], F32, tag="cntsb", name="cntsb")
                for j in range(NT):
                    tr_ps = misc_ps((128, E), "ntr")
                    nc.tensor.matmul(tr_ps[:],
                                     noisy_cm[:, j * 128:(j + 1) * 128],
                                     eye_sb[0:E, 0:E], is_transpose=True,
                                     start=True, stop=True)
                    nc.vector.tensor_copy(noisy8[:, 8 * j:8 * j + E], tr_ps[:])
                for j in range(NT):
                    nc.vector.max(m8[:, 8 * j:8 * j + 8],
                                  noisy8[:, 8 * j:8 * j + 8])
                m8v = m8[:].rearrange("p (j c) -> p j c", c=8)
                dall = spool.tile([128, NT], F32, tag="dall", name="dall",
                                  bufs=1)
                nc.vector.tensor_tensor(dall[:], m8v[:, :, 1], m8v[:, :, 0],
                                        ALU.subtract)
                # sigma(d) = 1/(1+exp(-d)) -- uses the exp table already
                # resident from attention (tanh would force a table swap
                # between the gelu loads)
                emd = spool.tile([128, NT], F32, tag="th", name="emd", bufs=1)
                nc.scalar.activation(emd[:], dall[:], AF.Exp, scale=-1.0)
                ope = spool.tile([128, NT], F32, tag="ope", name="ope", bufs=1)
                nc.vector.tensor_scalar_add(ope[:], emd[:], 1.0)
                spos = spool.tile([128, NT], F32, tag="spos", name="spos",
                                  bufs=1)
                nc.vector.reciprocal(spos[:], ope[:])
                sneg = spool.tile([128, NT], F32, tag="sneg", name="sneg",
                                  bufs=1)
                nc.vector.tensor_scalar(sneg[:], spos[:], -1.0, 1.0, ALU.mult,
                                        ALU.add)
                for j in range(NT):
                    nm = noisy8[:, 8 * j:8 * j + E]
                    v1 = m8[:, 8 * j:8 * j + 1]
                    v2 = m8[:, 8 * j + 1:8 * j + 2]
                    oh1 = spool.tile([128, E], F32, tag="oh1", name="oh1")
                    nc.vector.tensor_scalar(oh1[:], nm, v1, None, ALU.is_ge)
                    msk = mask[:, E * j:E * (j + 1)]
                    nc.vector.tensor_scalar(msk, nm, v2, None, ALU.is_ge)
                    oh2 = spool.tile([128, E], F32, tag="oh2", name="oh2")
                    nc.vector.tensor_tensor(oh2[:], msk, oh1[:], ALU.subtract)
                    g1 = spool.tile([128, E], F32, tag="gnum", name="g1")
                    nc.vector.tensor_scalar(g1[:], oh1[:],
                                            sneg[:, j:j + 1], None, ALU.mult)
                    g2 = spool.tile([128, E], F32, tag="gnum2", name="g2")
                    nc.vector.tensor_scalar(g2[:], oh2[:],
                                            spos[:, j:j + 1], None, ALU.mult)
                    nc.vector.tensor_tensor(gate[:, E * j:E * (j + 1)],
                                            g1[:], g2[:], ALU.add)
                    cps = misc_ps((1, E), "cnt")
                    _mm(nc, cps[:], ones_sb[:, 0:1], msk, True, True, F32)
                    nc.vector.tensor_copy(cnt_sb[0:1, E * j:E * (j + 1)], cps[:])

                # total counts -> all-gather
                tot = spool.tile([1, E], F32, tag="cnttot", name="cnttot",
                                 bufs=1)
                nc.vector.tensor_tensor(tot[:], cnt_sb[0:1, 0:E],
                                        cnt_sb[0:1, E:2 * E], ALU.add)
                nc.vector.tensor_tensor(tot[:], tot[:], cnt_sb[0:1, 2 * E:3 * E],
                                        ALU.add)
                nc.vector.tensor_tensor(tot[:], tot[:], cnt_sb[0:1, 3 * E:4 * E],
                                        ALU.add)
                nc.sync.dma_start(cnt_in[:], tot[:])
                nc.gpsimd.collective_compute(
                    "AllGather", ALU.bypass, replica_groups=rg_all,
                    ins=[cnt_in[:].opt()], outs=[cnt_out[:].opt()])

            # ---------- MLP hidden + MoE hidden (overlaps counts AG) ------
            Hm_sb = []
            for m in range(MLPH // 128):
                slab = wpool.tile([128, DT * 128], BF16, tag="qkslab",
                                  name="m1slab")
                nc.sync.dma_start(
                    slab[:], wmlp1_d[:, m * 1024:(m + 1) * 1024])
                ps = big_ps("hm")
                for kk in range(DT):
                    _mm(nc, ps[:], slab[:, kk * 128:(kk + 1) * 128],
                        x2T[kk][:], kk == 0, kk == DT - 1)
                hm = ppool.tile([128, TL], BF16, tag=f"hm{m}", name=f"hm{m}")
                nc.scalar.activation(hm[:], ps[:], AF.Gelu,
                                     bias=bmlp1_sb[:, m:m + 1])
                Hm_sb.append(hm)
            Hmoe = []
            for e in range(E):
                for hmi in range(MOEH // 128):
                    me = 2 * e + hmi
                    slab = wpool.tile([128, DT * 128], BF16, tag="qkslab",
                                      name="e1slab")
                    nc.sync.dma_start(
                        slab[:], we1_d[:, me * 1024:(me + 1) * 1024])
                    ps = big_ps("hmoe")
                    for kk in range(DT):
                        _mm(nc, ps[:], slab[:, kk * 128:(kk + 1) * 128],
                            x2T[kk][:], kk == 0, kk == DT - 1)
                    hs = ppool.tile([128, TL], BF16, tag=f"hmoe{me}",
                                    name=f"hmoe{me}")
                    nc.scalar.activation(
                        hs[:], ps[:], AF.Gelu,
                        bias=be1_sb[:, me:me + 1])
                    Hmoe.append(hs)

            # ---------- output GEMM part A: mlp2 (gate-independent) ------
            # runs before anything that needs the counts all-gather, so a
            # late cnt collective cannot stall the tensor queue behind it;
            # the xres residual is folded in here so part B only adds be2.
            mlp2_part = []
            for m in range(DT):
                slab = wpool.tile([128, 32 * 128], BF16, tag="outslabA",
                                  name="outslabA")
                nc.sync.dma_start(
                    slab[:], wout_d[:, m * 5120 + 1024:(m + 1) * 5120])
                ps = big_ps("outA")
                for kk in range(MLPH // 128):
                    _mm(nc, ps[:], slab[:, kk * 128:(kk + 1) * 128],
                        Hm_sb[kk][:], kk == 0, kk == MLPH // 128 - 1)
                pa = ppool.tile([128, TL], F32, tag=f"p2{m}", name=f"p2{m}")
                nc.vector.tensor_tensor(pa[:], ps[:], xres[m][:], ALU.add)
                mlp2_part.append(pa)

            # ---------- ranks / keep / gate_eff ----------
            cntg = spool.tile([NC, E], F32, tag="cntg", name="cntg", bufs=1)
            nc.sync.dma_start(cntg[:], cnt_out[:])
            off_ps = misc_ps((1, E), "off")
            _mm(nc, off_ps[:], wpfx_sb[:], cntg[:], True, True, F32)
            car = spool.tile([1, E * NT], F32, tag="car", name="car", bufs=1)
            nc.vector.tensor_copy(car[:, 0:E], off_ps[:])
            for j in range(1, NT):
                nc.vector.tensor_tensor(car[:, E * j:E * (j + 1)],
                                        car[:, E * (j - 1):E * j],
                                        cnt_sb[0:1, E * (j - 1):E * j],
                                        ALU.add)
            ge_tm = ppool.tile([128, E * NT], F32, tag="getm", name="getm")
            for j in range(NT):
                rk_ps = misc_ps((128, E), "rank")
                _mm(nc, rk_ps[:], utri_sb[:],
                    mask[:, E * j:E * (j + 1)], True, False, F32)
                _mm(nc, rk_ps[:], ones_sb[0:1, :],
                    car[:, E * j:E * (j + 1)], False, True, F32)
                keep = spool.tile([128, E], F32, tag="keep", name="keep")
                nc.vector.tensor_scalar(keep[:], rk_ps[:], float(CAP), None,
                                        ALU.is_lt)
                nc.vector.tensor_tensor(ge_tm[:, E * j:E * (j + 1)],
                                        gate[:, E * j:E * (j + 1)],
                                        keep[:], ALU.mult)
            for j in range(NT):
                tr_ps = misc_ps((E, 128), "getr")
                nc.tensor.matmul(tr_ps[:], ge_tm[:, E * j:E * (j + 1)],
                                 eye_sb[:, :], is_transpose=True,
                                 start=True, stop=True)
                nc.vector.tensor_copy(geT[:, j * 128:(j + 1) * 128], tr_ps[:])

            # gate the MoE hidden
            Hg = []
            for e in range(E):
                bc_ps = misc_ps((128, TL), "gbc")
                _mm(nc, bc_ps[:], gsel_sb[:, e * 128:(e + 1) * 128],
                    geT[:], True, True)
                bc_sb = spool.tile([128, TL], BF16, tag="gbcsb", name="gbcsb",
                                   bufs=2)
                nc.vector.tensor_copy(bc_sb[:], bc_ps[:])
                for hmi in range(MOEH // 128):
                    hg = ppool.tile([128, TL], BF16, tag=f"hg{2*e+hmi}",
                                    name=f"hg{2*e+hmi}")
                    nc.vector.tensor_tensor(hg[:], Hmoe[2 * e + hmi][:],
                                            bc_sb[:], ALU.mult)
                    Hg.append(hg)

            # ---------- output GEMM part B: moe + be2, add mlp2 partial --
            for m in range(DT):
                slab = wpool.tile([128, 8 * 128], BF16, tag="outslabB",
                                  name="outslabB")
                nc.sync.dma_start(
                    slab[:], wout_d[:, m * 5120:m * 5120 + 1024])
                ps = big_ps("out")
                for i8 in range(8):          # we2 tiles (e, hmi)
                    _mm(nc, ps[:], slab[:, i8 * 128:(i8 + 1) * 128],
                        Hg[i8][:], i8 == 0, False)
                _mm(nc, ps[:], be2_sb[:, m * 128:(m + 1) * 128],
                    geT[:], False, True)
                o = spool.tile([128, TL], F32, tag="outsb", name="outsb",
                               bufs=2)
                nc.vector.scalar_tensor_tensor(
                    o[:], ps[:], bmlp2_sb[:, m:m + 1], mlp2_part[m][:],
                    ALU.add, ALU.add)
                nc.sync.dma_start(out_d[m * 128:(m + 1) * 128, :], o[:])

    nc.compile()
    return nc


def _tile_lhst(w, n_k, n_m):
    # w: [n_k*128, n_m*128] -> [128, n_m, n_k, 128] -> [128, n_m*n_k*128]
    kdim, mdim = w.shape
    return np.ascontiguousarray(
        w.reshape(n_k, 128, n_m, 128).transpose(1, 2, 0, 3)
        .reshape(128, n_m * n_k * 128))


def _cols(a, n):
    # [n*128] -> [128, n] with column j = chunk j
    return np.ascontiguousarray(
        np.asarray(a, np.float32).reshape(n, 128).T)


def _prep_inputs(inputs):
    f32 = lambda a: np.ascontiguousarray(np.asarray(a, np.float32))
    bf = lambda a: np.ascontiguousarray(
        np.asarray(a, np.float32).astype(ml_dtypes.bfloat16))
    f8 = lambda a: np.ascontiguousarray(
        np.asarray(a, np.float32).astype(ml_dtypes.float8_e4m3))
    x = f32(inputs["x"]).reshape(T, D)
    noise = f32(inputs["noise"]).reshape(T, E)
    w_qkv = np.asarray(inputs["w_qkv"], np.float32)
    wqkT = w_qkv[:2 * D].T                       # [D, 2048]
    wvT = w_qkv[2 * D:].T                        # [D, D]
    wprojT = np.asarray(inputs["w_proj"], np.float32).T
    we1 = np.asarray(inputs["we1"], np.float32)  # [E, D, MOEH]
    we2 = np.asarray(inputs["we2"], np.float32)  # [E, MOEH, D]
    wmlp1 = np.asarray(inputs["w_mlp1"], np.float32)   # [D, MLPH]
    wmlp2 = np.asarray(inputs["w_mlp2"], np.float32)   # [MLPH, D]

    # we1 slabs: m-index = e*2+hmi over [D, 256] each
    we1_flat = np.concatenate([we1[e] for e in range(E)], 1)  # [D, E*MOEH]
    # wout: per m, 8 we2 tiles (e,hmi) then 32 wmlp2 tiles
    we2_l = we2.reshape(E, 2, 128, DT, 128).transpose(2, 3, 0, 1, 4) \
        .reshape(128, DT, 8, 128)
    wm2_l = wmlp2.reshape(32, 128, DT, 128).transpose(1, 2, 0, 3)
    wout = np.concatenate([we2_l, wm2_l], 2).reshape(128, DT * 40 * 128)

    shared = dict(
        wqk_l=f8(_tile_lhst(wqkT, DT, 16)),
        wv_l=f8(np.ascontiguousarray(
            wvT.reshape(DT, 128, D).transpose(1, 0, 2).reshape(128, DT * D))),
        wproj_l=f8(_tile_lhst(wprojT, DT, DT)),
        wmlp1_l=bf(_tile_lhst(wmlp1, DT, 32)),
        we1_l=bf(_tile_lhst(we1_flat, DT, 8)),
        wout_l=bf(wout),
        wrn=bf(np.concatenate([f32(inputs["w_route"]),
                               f32(inputs["w_noise"])], 1)
               .reshape(DT, 128, 2 * E).transpose(1, 0, 2)
               .reshape(128, DT * 2 * E)),
        be2=f32(inputs["be2"]),
        ln1g=_cols(inputs["ln1_g"], DT),
        ln1b=_cols(inputs["ln1_b"], DT),
        ln2g=_cols(inputs["ln2_g"], DT),
        ln2b=_cols(inputs["ln2_b"], DT),
        bproj=_cols(inputs["b_proj"], DT),
        be1=_cols(inputs["be1"], E * MOEH // 128),
        bmlp1=_cols(inputs["b_mlp1"], MLPH // 128),
        bmlp2=_cols(inputs["b_mlp2"], DT),
        brb=np.ascontiguousarray(np.stack(
            [f32(inputs["b_route"]), f32(inputs["b_noise"])], 1)),
        ones128=np.ones((128, 128), np.float32),
        eye128=np.eye(128, dtype=np.float32),
        utri128=np.triu(np.ones((128, 128), np.float32), 1),
        gsel=np.repeat(np.eye(E, dtype=np.float32), 128, 1),
        gsel8=np.ascontiguousarray((np.arange(8)[:, None] == (
            2 * (np.arange(512) // 128) + (np.arange(512) % 128) // 64
        )[None, :]).astype(np.float32)),
        ones128b=np.ones((128, 1), ml_dtypes.bfloat16),
    )
    in_maps = []
    for c in range(NC):
        m = dict(shared)
        m["xT"] = bf(x[c * TL:(c + 1) * TL].T)
        m["noiseT"] = bf(noise[c * TL:(c + 1) * TL].T)
        m["wpfx"] = (np.arange(NC) < c).astype(np.float32).reshape(NC, 1)
        in_maps.append(m)
    return in_maps


def _run(inputs, trace=False):
    if "nc" not in _cache:
        _cache["nc"] = _build()
    nc = _cache["nc"]
    in_maps = _prep_inputs(inputs)
    res = run_bass_kernel_spmd(nc, in_maps, core_ids=list(range(NC)),
                               trace=trace)
    _cache["last_res"] = res
    shards = [res.results[c]["out"] for c in range(NC)]   # each [D, TL]
    out = np.concatenate([np.asarray(s, np.float32).T for s in shards],
                         0).reshape(B, N, D)
    return out.astype(np.float32), res.exec_time_ns


def kernel(**inputs):
    out, _ = _run(inputs, trace=False)
    return out

